# revision 5
# baseline (speedup 1.0000x reference)
"""AnchorAttention distributed Bass kernel for 8 TRN2 NeuronCores.

Reference computation (B=2, S=4096, D=1024, H=16, Dh=64, A=512):
  anchors = x[:, :A];  queries = x[:, A:]
  anchor_q/k/v = split_heads(anchors @ Wq/Wk/Wv + b)
  query_q      = split_heads(queries @ Wqt + bqt)
  combined_q   = concat([anchor_q, query_q], axis=2)       # [B,H,S,Dh]
  out  = softmax(combined_q @ anchor_k^T / sqrt(Dh)) @ anchor_v
  out  = merge_heads(out) @ Wo + bo

Sharding: the B*S = 8192 token rows are split into 8 chunks of 1024 rows
(core c -> batch c//4, rows (c%4)*1024). Each core duplicates its
batch's anchor K/V projections, computes Q for its own rows, attention
over the 512 anchors for all 16 heads, and the output projection for its
rows. The output is a pure concatenation: no collectives.

Bias algebra (host-side):
  * bk is dropped entirely: adding bk to K shifts every anchor's score
    for a given row by the same amount, and softmax is shift-invariant.
  * bv is folded into bo: softmax rows sum to 1, so attn @ (V + bv) =
    attn @ V + bv, and (out + bv) @ Wo + bo = out @ Wo + (bv @ Wo + bo).

Layout: everything is kept transposed ([feature, row]) so each matmul
contracts over the partition dim with zero on-chip transposes; the final
output projection naturally lands un-transposed [row, feature] for DMA
out. Host pre-transposes/pre-casts inputs to bf16 (compute dtype; f32
accumulation in PSUM). Softmax row-sums come free via an extra all-ones
column appended to V; no max-subtraction is needed (scores are ~N(0,1),
exp stays in a tiny range; softmax is shift-invariant so results match).

Schedule (the big difference vs the naive phase ordering): input DMAs are
issued in consumption order with xt/wlo split per contraction slice, and
the Q projection runs dt-outer over all 8 PSUM banks so the PE starts as
soon as the first 512KB lands instead of waiting for whole slabs. The
K and V projections are folded INTO the attention pair pipeline (one
head-pair's worth per iteration, one iteration ahead), so their PE work
overlaps the scalar-engine Exp activations, which are the per-pair
bottleneck otherwise.

Heads are packed two per 128-partition tile (head h -> column-tile h//2,
partitions (h%2)*64 ..). Attention is software-pipelined over the 8 head
pairs: scores+exp run one pair ahead of AV, two ahead of normalization.
AV appends an all-ones V column so softmax sums fall out of the matmul;
1/sums (fast DVE reciprocal, lane-parallel at partition bases 0/64) is
partition-broadcast by a PE ones-outer-product written into already-
evacuated rows of the pair's second PSUM tile, and a single mixed-
partition-base DVE multiply writes the normalized attn^T slab.
"""

from contextlib import ExitStack

import numpy as np
import ml_dtypes

import concourse.bass as bass
import concourse.tile as tile
from concourse import bacc, mybir
from concourse import bass_utils

BF16 = mybir.dt.bfloat16
F32 = mybir.dt.float32
B, S, D = 2, 4096, 1024
H, DH = 16, 64
A = 512                  # num_anchor_tokens (asserted at runtime)
RPC = 1024               # rows per core
NCORES = 8
SCALE = 1.0 / np.sqrt(float(DH))

_CACHE = {}


def _build():
    """Build + compile the per-core Bass graph (identical on all cores)."""
    nc = bacc.Bacc("TRN2", target_bir_lowering=False, debug=False)

    xt = nc.dram_tensor("xt", [128, 8, RPC], BF16, kind="ExternalInput")   # rows^T swizzled
    at = nc.dram_tensor("at", [128, 8, A], BF16, kind="ExternalInput")     # anchors^T swizzled
    wlo = nc.dram_tensor("wlo", [128, 8, D], BF16, kind="ExternalInput")   # Q weight rows 0-511
    whi = nc.dram_tensor("whi", [128, 8, D], BF16, kind="ExternalInput")   # Q weight rows 512-1023
    wk = nc.dram_tensor("wk", [128, 8, 8, 128], BF16, kind="ExternalInput")  # pair-major
    wv = nc.dram_tensor("wv", [128, 8, 8, 128], BF16, kind="ExternalInput")  # pair-major
    wo = nc.dram_tensor("wo", [128, 8, D], BF16, kind="ExternalInput")
    blo = nc.dram_tensor("blo", [128, 8], F32, kind="ExternalInput")
    bhi = nc.dram_tensor("bhi", [128, 8], F32, kind="ExternalInput")
    bo = nc.dram_tensor("bo", [128, D], F32, kind="ExternalInput")   # pre-broadcast bo + bv@Wo

    out = nc.dram_tensor("out", [RPC, D], BF16, kind="ExternalOutput")

    Exp = mybir.ActivationFunctionType.Exp

    with tile.TileContext(nc) as tc:
        with tc.tile_pool(name="wpool", bufs=1) as wpool, \
             tc.tile_pool(name="cpool", bufs=1) as cpool, \
             tc.tile_pool(name="kvpool", bufs=1) as kvpool, \
             tc.tile_pool(name="qtpool", bufs=2) as qtpool:
            # x + Q weights live only through the Q projection; their pools
            # (and the 8-bank Q PSUM pool) close before the attention pools
            # open so the attention working set reuses their space.
            projstack = ExitStack()
            wqpool = projstack.enter_context(tc.tile_pool(name="wqpool", bufs=1))
            xpool = projstack.enter_context(tc.tile_pool(name="xpool", bufs=1))
            qpsum = projstack.enter_context(
                tc.tile_pool(name="qpsum", bufs=1, space="PSUM"))

            # ---- DMA issue order == consumption order. xt/wlo are split
            # per dt slice so the dt-outer Q projection starts on slice 0
            # while the rest stream in; everything later is whole-slab. ----
            blo_sb = cpool.tile([128, 8], F32, name="blo_sb")
            nc.sync.dma_start(out=blo_sb, in_=blo.ap())
            bhi_sb = cpool.tile([128, 8], F32, name="bhi_sb")
            nc.sync.dma_start(out=bhi_sb, in_=bhi.ap())

            xt_sb = xpool.tile([128, 8, RPC], BF16, name="xt_sb")
            wlo_sb = wqpool.tile([128, 8, D], BF16, name="wlo_sb")
            whi_sb = wqpool.tile([128, 8, D], BF16, name="whi_sb")
            for k in range(8):
                nc.sync.dma_start(out=xt_sb[:, k:k + 1, :],
                                  in_=xt.ap()[:, k:k + 1, :])
                nc.sync.dma_start(out=wlo_sb[:, k:k + 1, :],
                                  in_=wlo.ap()[:, k:k + 1, :])
            for k in range(8):
                nc.sync.dma_start(out=whi_sb[:, k:k + 1, :],
                                  in_=whi.ap()[:, k:k + 1, :])

            at_sb = wpool.tile([128, 8, A], BF16, name="at_sb")
            nc.sync.dma_start(out=at_sb, in_=at.ap())
            # K/V weights stream in pair-major slices: pair j's attention
            # iteration only needs slice j, so attention starts ~20us
            # earlier than waiting for whole slabs at ~185 GB/s.
            wk_sb = wpool.tile([128, 8, 8, 128], BF16, name="wk_sb")
            wv_sb = wpool.tile([128, 8, 8, 128], BF16, name="wv_sb")
            for j in range(8):
                nc.sync.dma_start(out=wk_sb[:, j:j + 1, :, :],
                                  in_=wk.ap()[:, j:j + 1, :, :])
                nc.sync.dma_start(out=wv_sb[:, j:j + 1, :, :],
                                  in_=wv.ap()[:, j:j + 1, :, :])
            wo_sb = wpool.tile([128, 8, D], BF16, name="wo_sb")
            nc.sync.dma_start(out=wo_sb, in_=wo.ap())
            bo_bc = cpool.tile([128, D], F32, name="bo_bc")
            nc.sync.dma_start(out=bo_bc, in_=bo.ap())

            ones_bf = cpool.tile([128, DH], BF16, name="ones_bf")
            nc.vector.memset(ones_bf, 1.0)

            # V slab: [128(a%128), ach, head, 65]; cols 0-63 = V head slice,
            # col 64 = ones (supplies softmax row-sums during AV).
            vaug = kvpool.tile([128, 4, H, DH + 1], BF16, name="vaug")
            nc.vector.memset(vaug, 1.0)
            kt_sb = kvpool.tile([128, 8, A], BF16, name="kt_sb")

            qtz = []
            for rc in range(2):
                qt_z0 = qtpool.tile([128, 8, 512], BF16, tag=f"qt0_{rc}",
                                    name=f"qt_z0_{rc}", bufs=1)
                qt_z1 = qtpool.tile([128, 8, 512], BF16, tag=f"qt1_{rc}",
                                    name=f"qt_z1_{rc}", bufs=1)
                nc.vector.memset(qt_z0[64:128, :, :], 0.0)
                nc.vector.memset(qt_z1[0:64, :, :], 0.0)
                qtz.append((qt_z0, qt_z1))

            # ---- Q^T projection per 512-row chunk, dt-OUTER across all 8
            # PSUM banks: matmuls for contraction slice dt only need DMA
            # slice dt of xt/wlo, so compute starts ~2MB earlier. Written
            # into two zero-padded slabs (z0: odd-head partitions zeroed,
            # z1: even) so score matmuls contract over the full 128
            # partitions (FWL stays on, no PE mode switches). ----
            for rc in range(2):
                wsel = wlo_sb if rc == 0 else whi_sb
                bsel = blo_sb if rc == 0 else bhi_sb
                qt_z0, qt_z1 = qtz[rc]
                pq = qpsum.tile([128, 8, 512], F32, tag="pq", name=f"pq{rc}")
                for dt in range(8):
                    for ct in range(8):
                        nc.tensor.matmul(
                            pq[:, ct, :], wsel[:, dt, ct * 128:(ct + 1) * 128],
                            xt_sb[:, dt, rc * 512:(rc + 1) * 512],
                            start=(dt == 0), stop=(dt == 7))
                for ct in range(8):
                    nc.vector.tensor_scalar_add(
                        qt_z0[0:64, ct, :], pq[0:64, ct, :], bsel[0:64, ct:ct + 1])
                    nc.vector.tensor_scalar_add(
                        qt_z1[64:128, ct, :], pq[64:128, ct, :],
                        bsel[64:128, ct:ct + 1])
            qts = qtz
            projstack.close()

            # ---- attention, software-pipelined over the 8 head-pair
            # groups (ct): scores+exp run one group ahead of AV, two ahead
            # of the normalization. The K and V projections for pair ct+1
            # run inside iteration ct (PE work that overlaps the scalar
            # Exp). Both heads of a group share one praw2 slab, one
            # reciprocal, and one [128, 1024] normalize multiply. The
            # 1/sums broadcast is a PE ones-outer-product written into
            # partitions 0-127 of the group's SECOND pav tile (its rows
            # were already evacuated), so no PSUM banks are added and the
            # DVE multiply reads it with mixed partition bases. ----
            attnstack = ExitStack()
            psum = attnstack.enter_context(
                tc.tile_pool(name="psum", bufs=2, space="PSUM"))
            attnpool = attnstack.enter_context(tc.tile_pool(name="attnpool", bufs=1))
            ptpool = attnstack.enter_context(tc.tile_pool(name="ptpool", bufs=8))
            tmppool = attnstack.enter_context(tc.tile_pool(name="tmppool", bufs=4))
            rcppool = attnstack.enter_context(tc.tile_pool(name="rcppool", bufs=3))
            outpool = attnstack.enter_context(tc.tile_pool(name="outpool", bufs=3))
            attnT = attnpool.tile([128, 8, RPC], BF16, name="attnT")

            def kvproj(j):
                # K^T and V projections for head pair j, one PSUM "work"
                # alloc (bank 0 = K^T pair slab, bank 1 = V [a,4x128]).
                kv = psum.tile([128, 2, 512], F32, tag="work", name="kv",
                               bufs=2)
                for dt in range(8):
                    nc.tensor.matmul(
                        kv[:, 0, :], wk_sb[:, j, dt, :],
                        at_sb[:, dt, :], start=(dt == 0), stop=(dt == 7))
                kvv = kv[:, 1, :].rearrange("p (a c) -> p a c", a=4)
                for ach in range(4):
                    for dt in range(8):
                        nc.tensor.matmul(
                            kvv[:, ach, :],
                            at_sb[:, dt, ach * 128:(ach + 1) * 128],
                            wv_sb[:, j, dt, :],
                            start=(dt == 0), stop=(dt == 7))
                nc.vector.tensor_copy(kt_sb[:, j, :], kv[:, 0, :])
                vsrc = kv[:, 1, :].rearrange("p (a h d) -> p a h d", a=4, h=2)
                nc.vector.tensor_copy(vaug[:, :, 2 * j:2 * j + 2, 0:DH], vsrc)

            def stage_scores(ct):
                st = {"pts": []}
                for par in range(2):
                    for rc in range(2):
                        qt_sb = qts[rc][par]
                        pt = ptpool.tile([128, 4, 512], BF16, tag="pt",
                                         name="pt")
                        for half in range(2):
                            s2 = psum.tile([128, 2, 512], F32, tag="s",
                                           name="s2", bufs=2)
                            for k in range(2):
                                ach = 2 * half + k
                                nc.tensor.matmul(
                                    s2[:, k, :],
                                    kt_sb[:, ct, ach * 128:(ach + 1) * 128],
                                    qt_sb[:, ct, :],
                                    start=True, stop=True)
                            nc.scalar.activation(
                                out=pt[:, 2 * half:2 * half + 2, :], in_=s2,
                                func=Exp, scale=SCALE)
                        st["pts"].append(pt)
                return st

            def stage_av(ct, par, st):
                h = 2 * ct + par
                pav = psum.tile([128, 2, 512], F32, tag="work", name="pav",
                                bufs=2)
                for rc in range(2):
                    pt = st["pts"][par * 2 + rc]
                    for ach in range(4):
                        nc.tensor.matmul(
                            pav[0:DH + 1, rc, :], vaug[:, ach, h, :],
                            pt[:, ach, :], start=(ach == 0), stop=(ach == 3))
                if par == 0:
                    st["praw2"] = tmppool.tile([128, 2, 512], BF16,
                                               tag="praw", name="praw2")
                    # sums gathered to partition bases {0,64} of one tile
                    # so the reciprocal+cast run lane-parallel
                    st["sums4"] = rcppool.tile([128, 2, 512], F32,
                                               tag="sums", name="sums4")
                nc.vector.tensor_copy(st["praw2"][par * 64:par * 64 + DH, :, :],
                                      pav[0:DH, :, :])
                row = par * 64
                nc.vector.tensor_copy(st["sums4"][row:row + 1, :, :],
                                      pav[DH:DH + 1, :, :])
                st[f"pav{par}"] = pav

            def stage_recip(ct, st):
                rcp4 = rcppool.tile([128, 2, 512], F32, tag="rcp",
                                    name="rcp4")
                nc.vector.reciprocal_approx_fast(rcp4, st["sums4"])
                rcpbf = rcppool.tile([128, 2, 512], BF16, tag="rcpbf",
                                     name="rcpbf")
                nc.vector.tensor_copy(rcpbf, rcp4)
                st["rcpbf"] = rcpbf

            def stage_norm(ct, st):
                pav1 = st["pav1"]
                for par in range(2):
                    row = par * 64
                    for rcn in range(2):
                        nc.tensor.matmul(
                            pav1[par * 64:(par + 1) * 64, rcn, :],
                            ones_bf[row:row + 1, :],
                            st["rcpbf"][row:row + 1, rcn, :],
                            start=True, stop=True)
                dst = attnT[:, ct, :].rearrange("p (b r) -> p b r", b=2)
                nc.vector.tensor_mul(dst, st["praw2"], pav1)

            # O-proj partials for the first two tiles are emitted inside
            # the pipeline drain so the PE has work while the last group's
            # normalization chain runs.
            pouts_head = []

            def oproj_partial():
                for nh in range(2):
                    pout = psum.tile([128, 512], F32, tag="work",
                                     name="pout")
                    for ct2 in range(7):
                        nc.tensor.matmul(
                            pout, attnT[:, ct2, 0:128],
                            wo_sb[:, ct2, nh * 512:(nh + 1) * 512],
                            start=(ct2 == 0), stop=False)
                    pouts_head.append(pout)

            kvproj(0)
            sts = {}
            for i in range(10):
                if i < 8:
                    sts[i] = stage_scores(i)
                if i + 1 < 8:
                    kvproj(i + 1)
                if i == 9:
                    oproj_partial()
                if 2 <= i <= 9:
                    stage_recip(i - 2, sts[i - 2])
                    stage_norm(i - 2, sts[i - 2])
                if 1 <= i <= 8:
                    stage_av(i - 1, 0, sts[i - 1])
                    stage_av(i - 1, 1, sts[i - 1])

            # ---- output projection ----
            for rti in range(8):
                for nh in range(2):
                    if rti == 0:
                        pout = pouts_head[nh]
                        nc.tensor.matmul(
                            pout, attnT[:, 7, 0:128],
                            wo_sb[:, 7, nh * 512:(nh + 1) * 512],
                            start=False, stop=True)
                    else:
                        pout = psum.tile([128, 512], F32, tag="work",
                                         name="pout")
                        for ct2 in range(8):
                            nc.tensor.matmul(
                                pout, attnT[:, ct2, rti * 128:(rti + 1) * 128],
                                wo_sb[:, ct2, nh * 512:(nh + 1) * 512],
                                start=(ct2 == 0), stop=(ct2 == 7))
                    out_t = outpool.tile([128, 512], BF16, tag="out",
                                         name="out_t")
                    nc.vector.tensor_add(out_t, pout,
                                         bo_bc[:, nh * 512:(nh + 1) * 512])
                    nc.sync.dma_start(
                        out=out.ap()[rti * 128:(rti + 1) * 128,
                                     nh * 512:(nh + 1) * 512],
                        in_=out_t)
            attnstack.close()

    nc.compile()
    return nc


def _swz(a):
    """[1024, cols] -> [128, 8, cols] with row r -> (r % 128, r // 128)."""
    return np.ascontiguousarray(
        a.reshape(8, 128, -1).transpose(1, 0, 2))


def _make_in_maps(x, Wq, bq, Wk, bk, Wv, bv, Wqt, bqt, Wo, bo):
    x = np.asarray(x, dtype=np.float32)
    bf = ml_dtypes.bfloat16

    wq_b = np.ascontiguousarray(np.asarray(Wq, np.float32).astype(bf))
    wqt_b = np.ascontiguousarray(np.asarray(Wqt, np.float32).astype(bf))
    wk_b = np.ascontiguousarray(np.asarray(Wk, np.float32).astype(bf))
    wv_b = np.ascontiguousarray(np.asarray(Wv, np.float32).astype(bf))
    wo_b = np.ascontiguousarray(np.asarray(Wo, np.float32).astype(bf))
    colmaj = lambda v: np.ascontiguousarray(
        np.asarray(v, np.float32).reshape(8, 128).T)
    bq, bqt = map(colmaj, (bq, bqt))
    # bv folded through Wo into the output bias (softmax rows sum to 1);
    # bk dropped (constant score shift per row, softmax-invariant).
    bo_eff = (np.asarray(bo, np.float32)
              + np.asarray(bv, np.float32) @ np.asarray(Wo, np.float32))
    bo_eff = np.ascontiguousarray(np.broadcast_to(bo_eff, (128, D)))

    wq_sw, wqt_sw = _swz(wq_b), _swz(wqt_b)
    pairmaj = lambda w: np.ascontiguousarray(
        _swz(w).reshape(128, 8, 8, 128).transpose(0, 2, 1, 3))
    wk_sw, wv_sw, wo_sw = pairmaj(wk_b), pairmaj(wv_b), _swz(wo_b)
    at_sw = [_swz(x[b, :A, :].T.astype(bf)) for b in range(B)]
    in_maps = []
    for c in range(NCORES):
        b, q = divmod(c, 4)
        rows = x[b, q * RPC:(q + 1) * RPC, :]
        in_maps.append({
            "xt": _swz(rows.T.astype(bf)),
            "at": at_sw[b],
            "wlo": wq_sw if q == 0 else wqt_sw,
            "whi": wqt_sw,
            "wk": wk_sw, "wv": wv_sw, "wo": wo_sw,
            "blo": bq if q == 0 else bqt, "bhi": bqt,
            "bo": bo_eff,
        })
    return in_maps


def kernel(x, Wq, bq, Wk, bk, Wv, bv, Wqt, bqt, Wo, bo, num_anchor_tokens):
    assert int(num_anchor_tokens) == A
    if "nc" not in _CACHE:
        _CACHE["nc"] = _build()
    nc = _CACHE["nc"]

    in_maps = _make_in_maps(x, Wq, bq, Wk, bk, Wv, bv, Wqt, bqt, Wo, bo)
    res = bass_utils.run_bass_kernel_spmd(
        nc, in_maps, core_ids=list(range(NCORES)))
    out = np.empty((B, S, D), np.float32)
    for c in range(NCORES):
        b, q = divmod(c, 4)
        out[b, q * RPC:(q + 1) * RPC, :] = res.results[c]["out"].astype(
            np.float32)
    return out


# revision 6
# speedup vs baseline: 1.0354x; 1.0354x over previous
"""AnchorAttention distributed Bass kernel for 8 TRN2 NeuronCores.

Reference computation (B=2, S=4096, D=1024, H=16, Dh=64, A=512):
  anchors = x[:, :A];  queries = x[:, A:]
  anchor_q/k/v = split_heads(anchors @ Wq/Wk/Wv + b)
  query_q      = split_heads(queries @ Wqt + bqt)
  combined_q   = concat([anchor_q, query_q], axis=2)       # [B,H,S,Dh]
  out  = softmax(combined_q @ anchor_k^T / sqrt(Dh)) @ anchor_v
  out  = merge_heads(out) @ Wo + bo

Sharding: the B*S = 8192 token rows are split into 8 chunks of 1024 rows
(core c -> batch c//4, rows (c%4)*1024). Each core duplicates its
batch's anchor K/V projections, computes Q for its own rows, attention
over the 512 anchors for all 16 heads, and the output projection for its
rows. The output is a pure concatenation: no collectives.

Bias algebra (host-side):
  * bk is dropped entirely: adding bk to K shifts every anchor's score
    for a given row by the same amount, and softmax is shift-invariant.
  * bv is folded into bo: softmax rows sum to 1, so attn @ (V + bv) =
    attn @ V + bv, and (out + bv) @ Wo + bo = out @ Wo + (bv @ Wo + bo).

Layout: everything is kept transposed ([feature, row]) so each matmul
contracts over the partition dim with zero on-chip transposes; the final
output projection naturally lands un-transposed [row, feature] for DMA
out. Host pre-transposes/pre-casts inputs to bf16 (compute dtype; f32
accumulation in PSUM). Softmax row-sums come free via an extra all-ones
column appended to V; no max-subtraction is needed (scores are ~N(0,1),
exp stays in a tiny range; softmax is shift-invariant so results match).

Schedule (the big difference vs the naive phase ordering): input DMAs are
issued in consumption order with xt/wlo split per contraction slice, and
the Q projection runs dt-outer over all 8 PSUM banks so the PE starts as
soon as the first 512KB lands instead of waiting for whole slabs. The
K and V projections are folded INTO the attention pair pipeline (one
head-pair's worth per iteration, one iteration ahead), so their PE work
overlaps the scalar-engine Exp activations, which are the per-pair
bottleneck otherwise.

Heads are packed two per 128-partition tile (head h -> column-tile h//2,
partitions (h%2)*64 ..). Attention is software-pipelined over the 8 head
pairs: scores+exp run one pair ahead of AV, two ahead of normalization.
AV appends an all-ones V column so softmax sums fall out of the matmul;
1/sums (fast DVE reciprocal, lane-parallel at partition bases 0/64) is
partition-broadcast by a PE ones-outer-product written into already-
evacuated rows of the pair's second PSUM tile, and a single mixed-
partition-base DVE multiply writes the normalized attn^T slab.
"""

from contextlib import ExitStack

import numpy as np
import ml_dtypes

import concourse.bass as bass
import concourse.tile as tile
from concourse import bacc, mybir
from concourse import bass_utils

BF16 = mybir.dt.bfloat16
F32 = mybir.dt.float32
B, S, D = 2, 4096, 1024
H, DH = 16, 64
A = 512                  # num_anchor_tokens (asserted at runtime)
RPC = 1024               # rows per core
NCORES = 8
SCALE = 1.0 / np.sqrt(float(DH))

_CACHE = {}


def _build():
    """Build + compile the per-core Bass graph (identical on all cores)."""
    nc = bacc.Bacc("TRN2", target_bir_lowering=False, debug=False)

    xt = nc.dram_tensor("xt", [128, 8, RPC], BF16, kind="ExternalInput")   # rows^T swizzled
    at = nc.dram_tensor("at", [128, 8, A], BF16, kind="ExternalInput")     # anchors^T swizzled
    wlo = nc.dram_tensor("wlo", [128, 8, D], BF16, kind="ExternalInput")   # Q weight rows 0-511
    whi = nc.dram_tensor("whi", [128, 8, D], BF16, kind="ExternalInput")   # Q weight rows 512-1023
    wk = nc.dram_tensor("wk", [128, 8, 8, 128], BF16, kind="ExternalInput")  # pair-major
    wv = nc.dram_tensor("wv", [128, 8, 8, 128], BF16, kind="ExternalInput")  # pair-major
    wo = nc.dram_tensor("wo", [128, 8, D], BF16, kind="ExternalInput")
    blo = nc.dram_tensor("blo", [128, 8], F32, kind="ExternalInput")
    bhi = nc.dram_tensor("bhi", [128, 8], F32, kind="ExternalInput")
    bo = nc.dram_tensor("bo", [128, D], F32, kind="ExternalInput")   # pre-broadcast bo + bv@Wo

    out = nc.dram_tensor("out", [RPC, D], BF16, kind="ExternalOutput")

    Exp = mybir.ActivationFunctionType.Exp

    with tile.TileContext(nc) as tc:
        with tc.tile_pool(name="wpool", bufs=1) as wpool, \
             tc.tile_pool(name="cpool", bufs=1) as cpool, \
             tc.tile_pool(name="kvpool", bufs=1) as kvpool, \
             tc.tile_pool(name="qtpool", bufs=2) as qtpool:
            # x + Q weights live only through the Q projection; their pools
            # (and the 8-bank Q PSUM pool) close before the attention pools
            # open so the attention working set reuses their space.
            projstack = ExitStack()
            wqpool = projstack.enter_context(tc.tile_pool(name="wqpool", bufs=1))
            xpool = projstack.enter_context(tc.tile_pool(name="xpool", bufs=1))
            qpsum = projstack.enter_context(
                tc.tile_pool(name="qpsum", bufs=1, space="PSUM"))

            # ---- DMA issue order == consumption order. xt/wlo are split
            # per dt slice so the dt-outer Q projection starts on slice 0
            # while the rest stream in; everything later is whole-slab. ----
            blo_sb = cpool.tile([128, 8], F32, name="blo_sb")
            nc.sync.dma_start(out=blo_sb, in_=blo.ap())
            bhi_sb = cpool.tile([128, 8], F32, name="bhi_sb")
            nc.sync.dma_start(out=bhi_sb, in_=bhi.ap())

            xt_sb = xpool.tile([128, 8, RPC], BF16, name="xt_sb")
            wlo_sb = wqpool.tile([128, 8, D], BF16, name="wlo_sb")
            whi_sb = wqpool.tile([128, 8, D], BF16, name="whi_sb")
            for k in range(8):
                nc.sync.dma_start(out=xt_sb[:, k:k + 1, :],
                                  in_=xt.ap()[:, k:k + 1, :])
                nc.sync.dma_start(out=wlo_sb[:, k:k + 1, :],
                                  in_=wlo.ap()[:, k:k + 1, :])
            for k in range(8):
                nc.sync.dma_start(out=whi_sb[:, k:k + 1, :],
                                  in_=whi.ap()[:, k:k + 1, :])

            at_sb = wpool.tile([128, 8, A], BF16, name="at_sb")
            nc.sync.dma_start(out=at_sb, in_=at.ap())
            # K/V weights stream in pair-major slices: pair j's attention
            # iteration only needs slice j, so attention starts ~20us
            # earlier than waiting for whole slabs at ~185 GB/s.
            wk_sb = wpool.tile([128, 8, 8, 128], BF16, name="wk_sb")
            wv_sb = wpool.tile([128, 8, 8, 128], BF16, name="wv_sb")
            for j in range(8):
                nc.sync.dma_start(out=wk_sb[:, j:j + 1, :, :],
                                  in_=wk.ap()[:, j:j + 1, :, :])
                nc.sync.dma_start(out=wv_sb[:, j:j + 1, :, :],
                                  in_=wv.ap()[:, j:j + 1, :, :])
            wo_sb = wpool.tile([128, 8, D], BF16, name="wo_sb")
            nc.sync.dma_start(out=wo_sb, in_=wo.ap())
            bo_bc = cpool.tile([128, D], F32, name="bo_bc")
            nc.sync.dma_start(out=bo_bc, in_=bo.ap())

            ones_bf = cpool.tile([128, DH], BF16, name="ones_bf")
            nc.vector.memset(ones_bf, 1.0)

            # V slab: [128(a%128), ach, head, 65]; cols 0-63 = V head slice,
            # col 64 = ones (supplies softmax row-sums during AV).
            vaug = kvpool.tile([128, 4, H, DH + 1], BF16, name="vaug")
            nc.vector.memset(vaug, 1.0)
            kt_sb = kvpool.tile([128, 8, A], BF16, name="kt_sb")

            qtz = []
            for rc in range(2):
                qt_z0 = qtpool.tile([128, 8, 512], BF16, tag=f"qt0_{rc}",
                                    name=f"qt_z0_{rc}", bufs=1)
                qt_z1 = qtpool.tile([128, 8, 512], BF16, tag=f"qt1_{rc}",
                                    name=f"qt_z1_{rc}", bufs=1)
                nc.vector.memset(qt_z0[64:128, :, :], 0.0)
                nc.vector.memset(qt_z1[0:64, :, :], 0.0)
                qtz.append((qt_z0, qt_z1))

            # ---- Q^T projection per 512-row chunk, dt-OUTER across all 8
            # PSUM banks: matmuls for contraction slice dt only need DMA
            # slice dt of xt/wlo, so compute starts ~2MB earlier. Written
            # into two zero-padded slabs (z0: odd-head partitions zeroed,
            # z1: even) so score matmuls contract over the full 128
            # partitions (FWL stays on, no PE mode switches). ----
            Ident = mybir.ActivationFunctionType.Identity
            for rc in range(2):
                wsel = wlo_sb if rc == 0 else whi_sb
                bsel = blo_sb if rc == 0 else bhi_sb
                qt_z0, qt_z1 = qtz[rc]
                # 4-bank half-passes, double-buffered: pass N+1's matmuls
                # overlap pass N's PSUM->SBUF evictions. The z0 eviction
                # rides the otherwise-idle scalar engine (Identity
                # activation with per-partition bias), z1 the vector
                # engine, so neither engine serializes the PE.
                for cg in range(2):
                    pq = qpsum.tile([128, 4, 512], F32, tag="pq", name="pq",
                                    bufs=2)
                    for dt in range(8):
                        for ci in range(4):
                            ct = cg * 4 + ci
                            nc.tensor.matmul(
                                pq[:, ci, :],
                                wsel[:, dt, ct * 128:(ct + 1) * 128],
                                xt_sb[:, dt, rc * 512:(rc + 1) * 512],
                                start=(dt == 0), stop=(dt == 7))
                    for ci in range(4):
                        ct = cg * 4 + ci
                        nc.scalar.activation(
                            out=qt_z0[0:64, ct, :], in_=pq[0:64, ci, :],
                            func=Ident, bias=bsel[0:64, ct:ct + 1])
                        nc.vector.tensor_scalar_add(
                            qt_z1[64:128, ct, :], pq[64:128, ci, :],
                            bsel[64:128, ct:ct + 1])
            qts = qtz
            projstack.close()

            # ---- attention, software-pipelined over the 8 head-pair
            # groups (ct): scores+exp run one group ahead of AV, two ahead
            # of the normalization. The K and V projections for pair ct+1
            # run inside iteration ct (PE work that overlaps the scalar
            # Exp). Both heads of a group share one praw2 slab, one
            # reciprocal, and one [128, 1024] normalize multiply. The
            # 1/sums broadcast is a PE ones-outer-product written into
            # partitions 0-127 of the group's SECOND pav tile (its rows
            # were already evacuated), so no PSUM banks are added and the
            # DVE multiply reads it with mixed partition bases. ----
            attnstack = ExitStack()
            psum = attnstack.enter_context(
                tc.tile_pool(name="psum", bufs=2, space="PSUM"))
            attnpool = attnstack.enter_context(tc.tile_pool(name="attnpool", bufs=1))
            ptpool = attnstack.enter_context(tc.tile_pool(name="ptpool", bufs=8))
            tmppool = attnstack.enter_context(tc.tile_pool(name="tmppool", bufs=4))
            rcppool = attnstack.enter_context(tc.tile_pool(name="rcppool", bufs=3))
            outpool = attnstack.enter_context(tc.tile_pool(name="outpool", bufs=3))
            attnT = attnpool.tile([128, 8, RPC], BF16, name="attnT")

            def kvproj(j):
                # K^T and V projections for head pair j, one PSUM "work"
                # alloc (bank 0 = K^T pair slab, bank 1 = V [a,4x128]).
                kv = psum.tile([128, 2, 512], F32, tag="work", name="kv",
                               bufs=2)
                for dt in range(8):
                    nc.tensor.matmul(
                        kv[:, 0, :], wk_sb[:, j, dt, :],
                        at_sb[:, dt, :], start=(dt == 0), stop=(dt == 7))
                kvv = kv[:, 1, :].rearrange("p (a c) -> p a c", a=4)
                for ach in range(4):
                    for dt in range(8):
                        nc.tensor.matmul(
                            kvv[:, ach, :],
                            at_sb[:, dt, ach * 128:(ach + 1) * 128],
                            wv_sb[:, j, dt, :],
                            start=(dt == 0), stop=(dt == 7))
                nc.vector.tensor_copy(kt_sb[:, j, :], kv[:, 0, :])
                vsrc = kv[:, 1, :].rearrange("p (a h d) -> p a h d", a=4, h=2)
                nc.vector.tensor_copy(vaug[:, :, 2 * j:2 * j + 2, 0:DH], vsrc)

            def stage_scores(ct):
                st = {"pts": []}
                for par in range(2):
                    for rc in range(2):
                        qt_sb = qts[rc][par]
                        pt = ptpool.tile([128, 4, 512], BF16, tag="pt",
                                         name="pt")
                        for half in range(2):
                            s2 = psum.tile([128, 2, 512], F32, tag="s",
                                           name="s2", bufs=2)
                            for k in range(2):
                                ach = 2 * half + k
                                nc.tensor.matmul(
                                    s2[:, k, :],
                                    kt_sb[:, ct, ach * 128:(ach + 1) * 128],
                                    qt_sb[:, ct, :],
                                    start=True, stop=True)
                            nc.scalar.activation(
                                out=pt[:, 2 * half:2 * half + 2, :], in_=s2,
                                func=Exp, scale=SCALE)
                        st["pts"].append(pt)
                return st

            def stage_av(ct, par, st):
                h = 2 * ct + par
                pav = psum.tile([128, 2, 512], F32, tag="work", name="pav",
                                bufs=2)
                for rc in range(2):
                    pt = st["pts"][par * 2 + rc]
                    for ach in range(4):
                        nc.tensor.matmul(
                            pav[0:DH + 1, rc, :], vaug[:, ach, h, :],
                            pt[:, ach, :], start=(ach == 0), stop=(ach == 3))
                if par == 0:
                    st["praw2"] = tmppool.tile([128, 2, 512], BF16,
                                               tag="praw", name="praw2")
                    # sums gathered to partition bases {0,64} of one tile
                    # so the reciprocal+cast run lane-parallel
                    st["sums4"] = rcppool.tile([128, 2, 512], F32,
                                               tag="sums", name="sums4")
                nc.vector.tensor_copy(st["praw2"][par * 64:par * 64 + DH, :, :],
                                      pav[0:DH, :, :])
                row = par * 64
                nc.vector.tensor_copy(st["sums4"][row:row + 1, :, :],
                                      pav[DH:DH + 1, :, :])
                st[f"pav{par}"] = pav

            def stage_recip(ct, st):
                rcp4 = rcppool.tile([128, 2, 512], F32, tag="rcp",
                                    name="rcp4")
                nc.vector.reciprocal_approx_fast(rcp4, st["sums4"])
                rcpbf = rcppool.tile([128, 2, 512], BF16, tag="rcpbf",
                                     name="rcpbf")
                nc.vector.tensor_copy(rcpbf, rcp4)
                st["rcpbf"] = rcpbf

            def stage_norm(ct, st):
                pav1 = st["pav1"]
                for par in range(2):
                    row = par * 64
                    for rcn in range(2):
                        nc.tensor.matmul(
                            pav1[par * 64:(par + 1) * 64, rcn, :],
                            ones_bf[row:row + 1, :],
                            st["rcpbf"][row:row + 1, rcn, :],
                            start=True, stop=True)
                dst = attnT[:, ct, :].rearrange("p (b r) -> p b r", b=2)
                nc.vector.tensor_mul(dst, st["praw2"], pav1)

            # O-proj partials for the first two tiles are emitted inside
            # the pipeline drain so the PE has work while the last group's
            # normalization chain runs.
            pouts_head = []

            def oproj_partial():
                for nh in range(2):
                    pout = psum.tile([128, 512], F32, tag="work",
                                     name="pout")
                    for ct2 in range(7):
                        nc.tensor.matmul(
                            pout, attnT[:, ct2, 0:128],
                            wo_sb[:, ct2, nh * 512:(nh + 1) * 512],
                            start=(ct2 == 0), stop=False)
                    pouts_head.append(pout)

            kvproj(0)
            sts = {}
            for i in range(10):
                if i < 8:
                    sts[i] = stage_scores(i)
                if i + 1 < 8:
                    kvproj(i + 1)
                if i == 9:
                    oproj_partial()
                if 2 <= i <= 9:
                    stage_recip(i - 2, sts[i - 2])
                    stage_norm(i - 2, sts[i - 2])
                if 1 <= i <= 8:
                    stage_av(i - 1, 0, sts[i - 1])
                    stage_av(i - 1, 1, sts[i - 1])

            # ---- output projection ----
            for rti in range(8):
                for nh in range(2):
                    if rti == 0:
                        pout = pouts_head[nh]
                        nc.tensor.matmul(
                            pout, attnT[:, 7, 0:128],
                            wo_sb[:, 7, nh * 512:(nh + 1) * 512],
                            start=False, stop=True)
                    else:
                        pout = psum.tile([128, 512], F32, tag="work",
                                         name="pout")
                        for ct2 in range(8):
                            nc.tensor.matmul(
                                pout, attnT[:, ct2, rti * 128:(rti + 1) * 128],
                                wo_sb[:, ct2, nh * 512:(nh + 1) * 512],
                                start=(ct2 == 0), stop=(ct2 == 7))
                    out_t = outpool.tile([128, 512], BF16, tag="out",
                                         name="out_t")
                    nc.vector.tensor_add(out_t, pout,
                                         bo_bc[:, nh * 512:(nh + 1) * 512])
                    nc.sync.dma_start(
                        out=out.ap()[rti * 128:(rti + 1) * 128,
                                     nh * 512:(nh + 1) * 512],
                        in_=out_t)
            attnstack.close()

    nc.compile()
    return nc


def _swz(a):
    """[1024, cols] -> [128, 8, cols] with row r -> (r % 128, r // 128)."""
    return np.ascontiguousarray(
        a.reshape(8, 128, -1).transpose(1, 0, 2))


def _make_in_maps(x, Wq, bq, Wk, bk, Wv, bv, Wqt, bqt, Wo, bo):
    x = np.asarray(x, dtype=np.float32)
    bf = ml_dtypes.bfloat16

    wq_b = np.ascontiguousarray(np.asarray(Wq, np.float32).astype(bf))
    wqt_b = np.ascontiguousarray(np.asarray(Wqt, np.float32).astype(bf))
    wk_b = np.ascontiguousarray(np.asarray(Wk, np.float32).astype(bf))
    wv_b = np.ascontiguousarray(np.asarray(Wv, np.float32).astype(bf))
    wo_b = np.ascontiguousarray(np.asarray(Wo, np.float32).astype(bf))
    colmaj = lambda v: np.ascontiguousarray(
        np.asarray(v, np.float32).reshape(8, 128).T)
    bq, bqt = map(colmaj, (bq, bqt))
    # bv folded through Wo into the output bias (softmax rows sum to 1);
    # bk dropped (constant score shift per row, softmax-invariant).
    bo_eff = (np.asarray(bo, np.float32)
              + np.asarray(bv, np.float32) @ np.asarray(Wo, np.float32))
    bo_eff = np.ascontiguousarray(np.broadcast_to(bo_eff, (128, D)))

    wq_sw, wqt_sw = _swz(wq_b), _swz(wqt_b)
    pairmaj = lambda w: np.ascontiguousarray(
        _swz(w).reshape(128, 8, 8, 128).transpose(0, 2, 1, 3))
    wk_sw, wv_sw, wo_sw = pairmaj(wk_b), pairmaj(wv_b), _swz(wo_b)
    at_sw = [_swz(x[b, :A, :].T.astype(bf)) for b in range(B)]
    in_maps = []
    for c in range(NCORES):
        b, q = divmod(c, 4)
        rows = x[b, q * RPC:(q + 1) * RPC, :]
        in_maps.append({
            "xt": _swz(rows.T.astype(bf)),
            "at": at_sw[b],
            "wlo": wq_sw if q == 0 else wqt_sw,
            "whi": wqt_sw,
            "wk": wk_sw, "wv": wv_sw, "wo": wo_sw,
            "blo": bq if q == 0 else bqt, "bhi": bqt,
            "bo": bo_eff,
        })
    return in_maps


def kernel(x, Wq, bq, Wk, bk, Wv, bv, Wqt, bqt, Wo, bo, num_anchor_tokens):
    assert int(num_anchor_tokens) == A
    if "nc" not in _CACHE:
        _CACHE["nc"] = _build()
    nc = _CACHE["nc"]

    in_maps = _make_in_maps(x, Wq, bq, Wk, bk, Wv, bv, Wqt, bqt, Wo, bo)
    res = bass_utils.run_bass_kernel_spmd(
        nc, in_maps, core_ids=list(range(NCORES)))
    out = np.empty((B, S, D), np.float32)
    for c in range(NCORES):
        b, q = divmod(c, 4)
        out[b, q * RPC:(q + 1) * RPC, :] = res.results[c]["out"].astype(
            np.float32)
    return out


# revision 7
# speedup vs baseline: 1.0615x; 1.0252x over previous
"""AnchorAttention distributed Bass kernel for 8 TRN2 NeuronCores.

Reference computation (B=2, S=4096, D=1024, H=16, Dh=64, A=512):
  anchors = x[:, :A];  queries = x[:, A:]
  anchor_q/k/v = split_heads(anchors @ Wq/Wk/Wv + b)
  query_q      = split_heads(queries @ Wqt + bqt)
  combined_q   = concat([anchor_q, query_q], axis=2)       # [B,H,S,Dh]
  out  = softmax(combined_q @ anchor_k^T / sqrt(Dh)) @ anchor_v
  out  = merge_heads(out) @ Wo + bo

Sharding: the B*S = 8192 token rows are split into 8 chunks of 1024 rows
(core c -> batch c//4, rows (c%4)*1024). Each core duplicates its
batch's anchor K/V projections, computes Q for its own rows, attention
over the 512 anchors for all 16 heads, and the output projection for its
rows. The output is a pure concatenation: no collectives.

Bias algebra (host-side):
  * bk is dropped entirely: adding bk to K shifts every anchor's score
    for a given row by the same amount, and softmax is shift-invariant.
  * bv is folded into bo: softmax rows sum to 1, so attn @ (V + bv) =
    attn @ V + bv, and (out + bv) @ Wo + bo = out @ Wo + (bv @ Wo + bo).

Layout: everything is kept transposed ([feature, row]) so each matmul
contracts over the partition dim with zero on-chip transposes; the final
output projection naturally lands un-transposed [row, feature] for DMA
out. Host pre-transposes/pre-casts inputs to bf16 (compute dtype; f32
accumulation in PSUM). Softmax row-sums come free via an extra all-ones
column appended to V; no max-subtraction is needed (scores are ~N(0,1),
exp stays in a tiny range; softmax is shift-invariant so results match).

Schedule (the big difference vs the naive phase ordering): input DMAs are
issued in consumption order with xt/wlo split per contraction slice, and
the Q projection runs dt-outer over all 8 PSUM banks so the PE starts as
soon as the first 512KB lands instead of waiting for whole slabs. The
K and V projections are folded INTO the attention pair pipeline (one
head-pair's worth per iteration, one iteration ahead), so their PE work
overlaps the scalar-engine Exp activations, which are the per-pair
bottleneck otherwise.

Heads are packed two per 128-partition tile (head h -> column-tile h//2,
partitions (h%2)*64 ..). Attention is software-pipelined over the 8 head
pairs: scores+exp run one pair ahead of AV, two ahead of normalization.
AV appends an all-ones V column so softmax sums fall out of the matmul;
1/sums (fast DVE reciprocal, lane-parallel at partition bases 0/64) is
partition-broadcast by a PE ones-outer-product written into already-
evacuated rows of the pair's second PSUM tile, and a single mixed-
partition-base DVE multiply writes the normalized attn^T slab.
"""

from contextlib import ExitStack

import numpy as np
import ml_dtypes

import concourse.bass as bass
import concourse.tile as tile
from concourse import bacc, mybir
from concourse import bass_utils

BF16 = mybir.dt.bfloat16
F32 = mybir.dt.float32
B, S, D = 2, 4096, 1024
H, DH = 16, 64
A = 512                  # num_anchor_tokens (asserted at runtime)
RPC = 1024               # rows per core
NCORES = 8
SCALE = 1.0 / np.sqrt(float(DH))

_CACHE = {}


def _build():
    """Build + compile the per-core Bass graph (identical on all cores)."""
    nc = bacc.Bacc("TRN2", target_bir_lowering=False, debug=False)

    xt = nc.dram_tensor("xt", [128, 8, RPC], BF16, kind="ExternalInput")   # rows^T swizzled
    at = nc.dram_tensor("at", [128, 8, A], BF16, kind="ExternalInput")     # anchors^T swizzled
    wlo = nc.dram_tensor("wlo", [128, 8, D], BF16, kind="ExternalInput")   # Q weight rows 0-511
    whi = nc.dram_tensor("whi", [128, 8, D], BF16, kind="ExternalInput")   # Q weight rows 512-1023
    wk = nc.dram_tensor("wk", [128, 8, 8, 128], BF16, kind="ExternalInput")  # pair-major
    wv = nc.dram_tensor("wv", [128, 8, 8, 128], BF16, kind="ExternalInput")  # pair-major
    wo = nc.dram_tensor("wo", [128, 8, D], BF16, kind="ExternalInput")
    blo = nc.dram_tensor("blo", [128, 8], F32, kind="ExternalInput")
    bhi = nc.dram_tensor("bhi", [128, 8], F32, kind="ExternalInput")
    bo = nc.dram_tensor("bo", [128, D], F32, kind="ExternalInput")   # pre-broadcast bo + bv@Wo

    out = nc.dram_tensor("out", [RPC, D], BF16, kind="ExternalOutput")

    Exp = mybir.ActivationFunctionType.Exp

    with tile.TileContext(nc) as tc:
        with tc.tile_pool(name="wpool", bufs=1) as wpool, \
             tc.tile_pool(name="cpool", bufs=1) as cpool, \
             tc.tile_pool(name="kvpool", bufs=1) as kvpool, \
             tc.tile_pool(name="qtpool", bufs=2) as qtpool:
            # x + Q weights live only through the Q projection; their pools
            # (and the 8-bank Q PSUM pool) close before the attention pools
            # open so the attention working set reuses their space.
            projstack = ExitStack()
            wqpool = projstack.enter_context(tc.tile_pool(name="wqpool", bufs=1))
            xpool = projstack.enter_context(tc.tile_pool(name="xpool", bufs=1))
            qpsum = projstack.enter_context(
                tc.tile_pool(name="qpsum", bufs=1, space="PSUM"))

            # ---- DMA issue order == consumption order. xt/wlo are split
            # per dt slice so the dt-outer Q projection starts on slice 0
            # while the rest stream in; everything later is whole-slab. ----
            blo_sb = cpool.tile([128, 8], F32, name="blo_sb")
            nc.sync.dma_start(out=blo_sb, in_=blo.ap())
            bhi_sb = cpool.tile([128, 8], F32, name="bhi_sb")
            nc.sync.dma_start(out=bhi_sb, in_=bhi.ap())

            xt_sb = xpool.tile([128, 8, RPC], BF16, name="xt_sb")
            wlo_sb = wqpool.tile([128, 8, D], BF16, name="wlo_sb")
            whi_sb = wqpool.tile([128, 8, D], BF16, name="whi_sb")
            for k in range(8):
                nc.sync.dma_start(out=xt_sb[:, k:k + 1, :],
                                  in_=xt.ap()[:, k:k + 1, :])
                nc.sync.dma_start(out=wlo_sb[:, k:k + 1, :],
                                  in_=wlo.ap()[:, k:k + 1, :])
            for k in range(8):
                nc.sync.dma_start(out=whi_sb[:, k:k + 1, :],
                                  in_=whi.ap()[:, k:k + 1, :])

            at_sb = wpool.tile([128, 8, A], BF16, name="at_sb")
            nc.sync.dma_start(out=at_sb, in_=at.ap())
            # K/V weights stream in pair-major slices: pair j's attention
            # iteration only needs slice j, so attention starts ~20us
            # earlier than waiting for whole slabs at ~185 GB/s.
            wk_sb = wpool.tile([128, 8, 8, 128], BF16, name="wk_sb")
            wv_sb = wpool.tile([128, 8, 8, 128], BF16, name="wv_sb")
            for j in range(8):
                nc.sync.dma_start(out=wk_sb[:, j:j + 1, :, :],
                                  in_=wk.ap()[:, j:j + 1, :, :])
                nc.sync.dma_start(out=wv_sb[:, j:j + 1, :, :],
                                  in_=wv.ap()[:, j:j + 1, :, :])
            wo_sb = wpool.tile([128, 8, D], BF16, name="wo_sb")
            nc.sync.dma_start(out=wo_sb, in_=wo.ap())
            bo_bc = cpool.tile([128, D], F32, name="bo_bc")
            nc.sync.dma_start(out=bo_bc, in_=bo.ap())

            ones_bf = cpool.tile([128, DH], BF16, name="ones_bf")
            nc.vector.memset(ones_bf, 1.0)

            # V slab: [128(a%128), ach, head, 65]; cols 0-63 = V head slice,
            # col 64 = ones (supplies softmax row-sums during AV).
            vaug = kvpool.tile([128, 4, H, DH + 1], BF16, name="vaug")
            nc.vector.memset(vaug, 1.0)
            kt_sb = kvpool.tile([128, 8, A], BF16, name="kt_sb")

            qtz = []
            for rc in range(2):
                qt_z0 = qtpool.tile([128, 8, 512], BF16, tag=f"qt0_{rc}",
                                    name=f"qt_z0_{rc}", bufs=1)
                qt_z1 = qtpool.tile([128, 8, 512], BF16, tag=f"qt1_{rc}",
                                    name=f"qt_z1_{rc}", bufs=1)
                for ct in range(8):
                    nc.vector.memset(qt_z0[64:128, ct, :], 0.0)
                    nc.vector.memset(qt_z1[0:64, ct, :], 0.0)
                qtz.append((qt_z0, qt_z1))

            # ---- Q^T projection per 512-row chunk, dt-OUTER across all 8
            # PSUM banks: matmuls for contraction slice dt only need DMA
            # slice dt of xt/wlo, so compute starts ~2MB earlier. Written
            # into two zero-padded slabs (z0: odd-head partitions zeroed,
            # z1: even) so score matmuls contract over the full 128
            # partitions (FWL stays on, no PE mode switches). ----
            Ident = mybir.ActivationFunctionType.Identity
            for rc in range(2):
                wsel = wlo_sb if rc == 0 else whi_sb
                bsel = blo_sb if rc == 0 else bhi_sb
                qt_z0, qt_z1 = qtz[rc]
                # 4-bank half-passes, double-buffered: pass N+1's matmuls
                # overlap pass N's PSUM->SBUF evictions. The z0 eviction
                # rides the otherwise-idle scalar engine (Identity
                # activation with per-partition bias), z1 the vector
                # engine, so neither engine serializes the PE.
                for cg in range(2):
                    pq = qpsum.tile([128, 4, 512], F32, tag="pq", name="pq",
                                    bufs=2)
                    for dt in range(8):
                        for ci in range(4):
                            ct = cg * 4 + ci
                            nc.tensor.matmul(
                                pq[:, ci, :],
                                wsel[:, dt, ct * 128:(ct + 1) * 128],
                                xt_sb[:, dt, rc * 512:(rc + 1) * 512],
                                start=(dt == 0), stop=(dt == 7))
                    for ci in range(4):
                        ct = cg * 4 + ci
                        nc.scalar.activation(
                            out=qt_z0[0:64, ct, :], in_=pq[0:64, ci, :],
                            func=Ident, bias=bsel[0:64, ct:ct + 1])
                        nc.vector.tensor_scalar_add(
                            qt_z1[64:128, ct, :], pq[64:128, ci, :],
                            bsel[64:128, ct:ct + 1])
            qts = qtz
            projstack.close()

            # ---- attention, software-pipelined over the 8 head-pair
            # groups (ct): scores+exp run one group ahead of AV, two ahead
            # of the normalization. The K and V projections for pair ct+1
            # run inside iteration ct (PE work that overlaps the scalar
            # Exp). Both heads of a group share one praw2 slab, one
            # reciprocal, and one [128, 1024] normalize multiply. The
            # 1/sums broadcast is a PE ones-outer-product written into
            # partitions 0-127 of the group's SECOND pav tile (its rows
            # were already evacuated), so no PSUM banks are added and the
            # DVE multiply reads it with mixed partition bases. ----
            attnstack = ExitStack()
            psum = attnstack.enter_context(
                tc.tile_pool(name="psum", bufs=2, space="PSUM"))
            attnpool = attnstack.enter_context(tc.tile_pool(name="attnpool", bufs=1))
            ptpool = attnstack.enter_context(tc.tile_pool(name="ptpool", bufs=8))
            tmppool = attnstack.enter_context(tc.tile_pool(name="tmppool", bufs=4))
            rcppool = attnstack.enter_context(tc.tile_pool(name="rcppool", bufs=3))
            outpool = attnstack.enter_context(tc.tile_pool(name="outpool", bufs=3))
            attnT = attnpool.tile([128, 8, RPC], BF16, name="attnT")

            def kvproj(j):
                # K^T and V projections for head pair j, one PSUM "work"
                # alloc (bank 0 = K^T pair slab, bank 1 = V [a,4x128]).
                kv = psum.tile([128, 2, 512], F32, tag="work", name="kv",
                               bufs=2)
                for dt in range(8):
                    nc.tensor.matmul(
                        kv[:, 0, :], wk_sb[:, j, dt, :],
                        at_sb[:, dt, :], start=(dt == 0), stop=(dt == 7))
                kvv = kv[:, 1, :].rearrange("p (a c) -> p a c", a=4)
                for ach in range(4):
                    for dt in range(8):
                        nc.tensor.matmul(
                            kvv[:, ach, :],
                            at_sb[:, dt, ach * 128:(ach + 1) * 128],
                            wv_sb[:, j, dt, :],
                            start=(dt == 0), stop=(dt == 7))
                nc.vector.tensor_copy(kt_sb[:, j, :], kv[:, 0, :])
                vsrc = kv[:, 1, :].rearrange("p (a h d) -> p a h d", a=4, h=2)
                nc.vector.tensor_copy(vaug[:, :, 2 * j:2 * j + 2, 0:DH], vsrc)

            def stage_scores(ct):
                st = {"pts": []}
                for par in range(2):
                    for rc in range(2):
                        qt_sb = qts[rc][par]
                        pt = ptpool.tile([128, 4, 512], BF16, tag="pt",
                                         name="pt")
                        for half in range(2):
                            s2 = psum.tile([128, 2, 512], F32, tag="s",
                                           name="s2", bufs=2)
                            for k in range(2):
                                ach = 2 * half + k
                                nc.tensor.matmul(
                                    s2[:, k, :],
                                    kt_sb[:, ct, ach * 128:(ach + 1) * 128],
                                    qt_sb[:, ct, :],
                                    start=True, stop=True)
                            nc.scalar.activation(
                                out=pt[:, 2 * half:2 * half + 2, :], in_=s2,
                                func=Exp, scale=SCALE)
                        st["pts"].append(pt)
                return st

            def stage_av(ct, par, st):
                h = 2 * ct + par
                pav = psum.tile([128, 2, 512], F32, tag="work", name="pav",
                                bufs=2)
                for rc in range(2):
                    pt = st["pts"][par * 2 + rc]
                    for ach in range(4):
                        nc.tensor.matmul(
                            pav[0:DH + 1, rc, :], vaug[:, ach, h, :],
                            pt[:, ach, :], start=(ach == 0), stop=(ach == 3))
                if par == 0:
                    st["praw2"] = tmppool.tile([128, 2, 512], BF16,
                                               tag="praw", name="praw2")
                    # sums gathered to partition bases {0,64} of one tile
                    # so the reciprocal+cast run lane-parallel
                    st["sums4"] = rcppool.tile([128, 2, 512], F32,
                                               tag="sums", name="sums4")
                nc.vector.tensor_copy(st["praw2"][par * 64:par * 64 + DH, :, :],
                                      pav[0:DH, :, :])
                row = par * 64
                nc.vector.tensor_copy(st["sums4"][row:row + 1, :, :],
                                      pav[DH:DH + 1, :, :])
                st[f"pav{par}"] = pav

            def stage_recip(ct, st):
                rcp4 = rcppool.tile([128, 2, 512], F32, tag="rcp",
                                    name="rcp4")
                nc.vector.reciprocal_approx_fast(rcp4, st["sums4"])
                rcpbf = rcppool.tile([128, 2, 512], BF16, tag="rcpbf",
                                     name="rcpbf")
                nc.vector.tensor_copy(rcpbf, rcp4)
                st["rcpbf"] = rcpbf

            def stage_norm(ct, st):
                pav1 = st["pav1"]
                for par in range(2):
                    row = par * 64
                    for rcn in range(2):
                        nc.tensor.matmul(
                            pav1[par * 64:(par + 1) * 64, rcn, :],
                            ones_bf[row:row + 1, :],
                            st["rcpbf"][row:row + 1, rcn, :],
                            start=True, stop=True)
                dst = attnT[:, ct, :].rearrange("p (b r) -> p b r", b=2)
                nc.vector.tensor_mul(dst, st["praw2"], pav1)

            # O-proj partials for the first two row-tiles are emitted
            # inside the pipeline drain so the PE has work while the last
            # groups' normalization chains run. They live in "s"-tag PSUM
            # (free once the last scores are exp'd); [:, nh, :] holds the
            # nh-th 512-col half.
            pouts_head = {}

            def oproj_partial(cts, rtis):
                for rti in rtis:
                    if rti not in pouts_head:
                        pouts_head[rti] = psum.tile(
                            [128, 2, 512], F32, tag="s", name="pout2",
                            bufs=2)
                    pout2 = pouts_head[rti]
                    for nh in range(2):
                        for ct2 in cts:
                            nc.tensor.matmul(
                                pout2[:, nh, :],
                                attnT[:, ct2, rti * 128:(rti + 1) * 128],
                                wo_sb[:, ct2, nh * 512:(nh + 1) * 512],
                                start=(ct2 == 0), stop=False)

            kvproj(0)
            sts = {}
            for i in range(10):
                if i < 8:
                    sts[i] = stage_scores(i)
                if i + 1 < 8:
                    kvproj(i + 1)
                if i == 8:
                    oproj_partial(range(6), [0, 1])
                if i == 9:
                    oproj_partial([6], [0, 1])
                if 2 <= i <= 9:
                    stage_recip(i - 2, sts[i - 2])
                    stage_norm(i - 2, sts[i - 2])
                if 1 <= i <= 8:
                    stage_av(i - 1, 0, sts[i - 1])
                    stage_av(i - 1, 1, sts[i - 1])

            # ---- output projection ----
            for rti in range(8):
                for nh in range(2):
                    if rti <= 1:
                        pout = pouts_head[rti][:, nh, :]
                        nc.tensor.matmul(
                            pout, attnT[:, 7, rti * 128:(rti + 1) * 128],
                            wo_sb[:, 7, nh * 512:(nh + 1) * 512],
                            start=False, stop=True)
                    else:
                        pout = psum.tile([128, 512], F32, tag="work",
                                         name="pout")
                        for ct2 in range(8):
                            nc.tensor.matmul(
                                pout, attnT[:, ct2, rti * 128:(rti + 1) * 128],
                                wo_sb[:, ct2, nh * 512:(nh + 1) * 512],
                                start=(ct2 == 0), stop=(ct2 == 7))
                    out_t = outpool.tile([128, 512], BF16, tag="out",
                                         name="out_t")
                    nc.vector.tensor_add(out_t, pout,
                                         bo_bc[:, nh * 512:(nh + 1) * 512])
                    nc.sync.dma_start(
                        out=out.ap()[rti * 128:(rti + 1) * 128,
                                     nh * 512:(nh + 1) * 512],
                        in_=out_t)
            attnstack.close()

    nc.compile()
    return nc


def _swz(a):
    """[1024, cols] -> [128, 8, cols] with row r -> (r % 128, r // 128)."""
    return np.ascontiguousarray(
        a.reshape(8, 128, -1).transpose(1, 0, 2))


def _make_in_maps(x, Wq, bq, Wk, bk, Wv, bv, Wqt, bqt, Wo, bo):
    x = np.asarray(x, dtype=np.float32)
    bf = ml_dtypes.bfloat16

    wq_b = np.ascontiguousarray(np.asarray(Wq, np.float32).astype(bf))
    wqt_b = np.ascontiguousarray(np.asarray(Wqt, np.float32).astype(bf))
    wk_b = np.ascontiguousarray(np.asarray(Wk, np.float32).astype(bf))
    wv_b = np.ascontiguousarray(np.asarray(Wv, np.float32).astype(bf))
    wo_b = np.ascontiguousarray(np.asarray(Wo, np.float32).astype(bf))
    colmaj = lambda v: np.ascontiguousarray(
        np.asarray(v, np.float32).reshape(8, 128).T)
    bq, bqt = map(colmaj, (bq, bqt))
    # bv folded through Wo into the output bias (softmax rows sum to 1);
    # bk dropped (constant score shift per row, softmax-invariant).
    bo_eff = (np.asarray(bo, np.float32)
              + np.asarray(bv, np.float32) @ np.asarray(Wo, np.float32))
    bo_eff = np.ascontiguousarray(np.broadcast_to(bo_eff, (128, D)))

    wq_sw, wqt_sw = _swz(wq_b), _swz(wqt_b)
    pairmaj = lambda w: np.ascontiguousarray(
        _swz(w).reshape(128, 8, 8, 128).transpose(0, 2, 1, 3))
    wk_sw, wv_sw, wo_sw = pairmaj(wk_b), pairmaj(wv_b), _swz(wo_b)
    at_sw = [_swz(x[b, :A, :].T.astype(bf)) for b in range(B)]
    in_maps = []
    for c in range(NCORES):
        b, q = divmod(c, 4)
        rows = x[b, q * RPC:(q + 1) * RPC, :]
        in_maps.append({
            "xt": _swz(rows.T.astype(bf)),
            "at": at_sw[b],
            "wlo": wq_sw if q == 0 else wqt_sw,
            "whi": wqt_sw,
            "wk": wk_sw, "wv": wv_sw, "wo": wo_sw,
            "blo": bq if q == 0 else bqt, "bhi": bqt,
            "bo": bo_eff,
        })
    return in_maps


def kernel(x, Wq, bq, Wk, bk, Wv, bv, Wqt, bqt, Wo, bo, num_anchor_tokens):
    assert int(num_anchor_tokens) == A
    if "nc" not in _CACHE:
        _CACHE["nc"] = _build()
    nc = _CACHE["nc"]

    in_maps = _make_in_maps(x, Wq, bq, Wk, bk, Wv, bv, Wqt, bqt, Wo, bo)
    res = bass_utils.run_bass_kernel_spmd(
        nc, in_maps, core_ids=list(range(NCORES)))
    out = np.empty((B, S, D), np.float32)
    for c in range(NCORES):
        b, q = divmod(c, 4)
        out[b, q * RPC:(q + 1) * RPC, :] = res.results[c]["out"].astype(
            np.float32)
    return out


# revision 8
# speedup vs baseline: 1.0703x; 1.0083x over previous
"""AnchorAttention distributed Bass kernel for 8 TRN2 NeuronCores.

Reference computation (B=2, S=4096, D=1024, H=16, Dh=64, A=512):
  anchors = x[:, :A];  queries = x[:, A:]
  anchor_q/k/v = split_heads(anchors @ Wq/Wk/Wv + b)
  query_q      = split_heads(queries @ Wqt + bqt)
  combined_q   = concat([anchor_q, query_q], axis=2)       # [B,H,S,Dh]
  out  = softmax(combined_q @ anchor_k^T / sqrt(Dh)) @ anchor_v
  out  = merge_heads(out) @ Wo + bo

Sharding: the B*S = 8192 token rows are split into 8 chunks of 1024 rows
(core c -> batch c//4, rows (c%4)*1024). Each core duplicates its
batch's anchor K/V projections, computes Q for its own rows, attention
over the 512 anchors for all 16 heads, and the output projection for its
rows. The output is a pure concatenation: no collectives.

Bias algebra (host-side):
  * bk is dropped entirely: adding bk to K shifts every anchor's score
    for a given row by the same amount, and softmax is shift-invariant.
  * bv is folded into bo: softmax rows sum to 1, so attn @ (V + bv) =
    attn @ V + bv, and (out + bv) @ Wo + bo = out @ Wo + (bv @ Wo + bo).

Layout: everything is kept transposed ([feature, row]) so each matmul
contracts over the partition dim with zero on-chip transposes; the final
output projection naturally lands un-transposed [row, feature] for DMA
out. Host pre-transposes/pre-casts inputs to bf16 (compute dtype; f32
accumulation in PSUM). Softmax row-sums come free via an extra all-ones
column appended to V; no max-subtraction is needed (scores are ~N(0,1),
exp stays in a tiny range; softmax is shift-invariant so results match).

Schedule (the big difference vs the naive phase ordering): input DMAs are
issued in consumption order with xt/wlo split per contraction slice, and
the Q projection runs dt-outer over all 8 PSUM banks so the PE starts as
soon as the first 512KB lands instead of waiting for whole slabs. The
K and V projections are folded INTO the attention pair pipeline (one
head-pair's worth per iteration, one iteration ahead), so their PE work
overlaps the scalar-engine Exp activations, which are the per-pair
bottleneck otherwise.

Heads are packed two per 128-partition tile (head h -> column-tile h//2,
partitions (h%2)*64 ..). Attention is software-pipelined over the 8 head
pairs: scores+exp run one pair ahead of AV, two ahead of normalization.
AV appends an all-ones V column so softmax sums fall out of the matmul;
1/sums (fast DVE reciprocal, lane-parallel at partition bases 0/64) is
partition-broadcast by a PE ones-outer-product written into already-
evacuated rows of the pair's second PSUM tile, and a single mixed-
partition-base DVE multiply writes the normalized attn^T slab.
"""

from contextlib import ExitStack

import numpy as np
import ml_dtypes

import concourse.bass as bass
import concourse.tile as tile
from concourse import bacc, mybir
from concourse import bass_utils

BF16 = mybir.dt.bfloat16
F32 = mybir.dt.float32
B, S, D = 2, 4096, 1024
H, DH = 16, 64
A = 512                  # num_anchor_tokens (asserted at runtime)
RPC = 1024               # rows per core
NCORES = 8
SCALE = 1.0 / np.sqrt(float(DH))

_CACHE = {}


def _build():
    """Build + compile the per-core Bass graph (identical on all cores)."""
    nc = bacc.Bacc("TRN2", target_bir_lowering=False, debug=False)

    xt = nc.dram_tensor("xt", [128, 8, RPC], BF16, kind="ExternalInput")   # rows^T swizzled
    at = nc.dram_tensor("at", [128, 8, A], BF16, kind="ExternalInput")     # anchors^T swizzled
    wlo = nc.dram_tensor("wlo", [128, 8, D], BF16, kind="ExternalInput")   # Q weight rows 0-511
    whi = nc.dram_tensor("whi", [128, 8, D], BF16, kind="ExternalInput")   # Q weight rows 512-1023
    wk = nc.dram_tensor("wk", [128, 8, 8, 128], BF16, kind="ExternalInput")  # pair-major
    wv = nc.dram_tensor("wv", [128, 8, 8, 128], BF16, kind="ExternalInput")  # pair-major
    wo = nc.dram_tensor("wo", [128, 8, D], BF16, kind="ExternalInput")
    blo = nc.dram_tensor("blo", [128, 8], F32, kind="ExternalInput")
    bhi = nc.dram_tensor("bhi", [128, 8], F32, kind="ExternalInput")
    bo = nc.dram_tensor("bo", [128, D], F32, kind="ExternalInput")   # pre-broadcast bo + bv@Wo

    out = nc.dram_tensor("out", [RPC, D], BF16, kind="ExternalOutput")

    Exp = mybir.ActivationFunctionType.Exp

    with tile.TileContext(nc) as tc:
        with tc.tile_pool(name="wpool", bufs=1) as wpool, \
             tc.tile_pool(name="cpool", bufs=1) as cpool, \
             tc.tile_pool(name="kvpool", bufs=1) as kvpool, \
             tc.tile_pool(name="qtpool", bufs=2) as qtpool:
            # x + Q weights live only through the Q projection; their pools
            # (and the 8-bank Q PSUM pool) close before the attention pools
            # open so the attention working set reuses their space.
            projstack = ExitStack()
            wqpool = projstack.enter_context(tc.tile_pool(name="wqpool", bufs=1))
            xpool = projstack.enter_context(tc.tile_pool(name="xpool", bufs=1))
            qpsum = projstack.enter_context(
                tc.tile_pool(name="qpsum", bufs=1, space="PSUM"))

            # ---- DMA issue order == consumption order. xt/wlo are split
            # per dt slice so the dt-outer Q projection starts on slice 0
            # while the rest stream in; everything later is whole-slab. ----
            blo_sb = cpool.tile([128, 8], F32, name="blo_sb")
            nc.sync.dma_start(out=blo_sb, in_=blo.ap())
            bhi_sb = cpool.tile([128, 8], F32, name="bhi_sb")
            nc.sync.dma_start(out=bhi_sb, in_=bhi.ap())

            xt_sb = xpool.tile([128, 8, RPC], BF16, name="xt_sb")
            wlo_sb = wqpool.tile([128, 8, D], BF16, name="wlo_sb")
            whi_sb = wqpool.tile([128, 8, D], BF16, name="whi_sb")
            for k in range(8):
                nc.sync.dma_start(out=xt_sb[:, k:k + 1, :],
                                  in_=xt.ap()[:, k:k + 1, :])
                nc.sync.dma_start(out=wlo_sb[:, k:k + 1, :],
                                  in_=wlo.ap()[:, k:k + 1, :])
            for k in range(8):
                nc.sync.dma_start(out=whi_sb[:, k:k + 1, :],
                                  in_=whi.ap()[:, k:k + 1, :])

            at_sb = wpool.tile([128, 8, A], BF16, name="at_sb")
            nc.sync.dma_start(out=at_sb, in_=at.ap())
            # K/V weights stream in pair-major slices: pair j's attention
            # iteration only needs slice j, so attention starts ~20us
            # earlier than waiting for whole slabs at ~185 GB/s.
            wk_sb = wpool.tile([128, 8, 8, 128], BF16, name="wk_sb")
            wv_sb = wpool.tile([128, 8, 8, 128], BF16, name="wv_sb")
            for j in range(8):
                nc.sync.dma_start(out=wk_sb[:, j:j + 1, :, :],
                                  in_=wk.ap()[:, j:j + 1, :, :])
                nc.sync.dma_start(out=wv_sb[:, j:j + 1, :, :],
                                  in_=wv.ap()[:, j:j + 1, :, :])
            wo_sb = wpool.tile([128, 8, D], BF16, name="wo_sb")
            nc.sync.dma_start(out=wo_sb, in_=wo.ap())
            bo_bc = cpool.tile([128, D], F32, name="bo_bc")
            nc.sync.dma_start(out=bo_bc, in_=bo.ap())

            ones_bf = cpool.tile([128, DH], BF16, name="ones_bf")
            nc.vector.memset(ones_bf, 1.0)

            # V slab: [128(a%128), ach, head, 65]; cols 0-63 = V head slice,
            # col 64 = ones (supplies softmax row-sums during AV).
            vaug = kvpool.tile([128, 4, H, DH + 1], BF16, name="vaug")
            nc.vector.memset(vaug, 1.0)
            kt_sb = kvpool.tile([128, 8, A], BF16, name="kt_sb")

            qtz = []
            for rc in range(2):
                qt_z0 = qtpool.tile([128, 8, 512], BF16, tag=f"qt0_{rc}",
                                    name=f"qt_z0_{rc}", bufs=1)
                qt_z1 = qtpool.tile([128, 8, 512], BF16, tag=f"qt1_{rc}",
                                    name=f"qt_z1_{rc}", bufs=1)
                for ct in range(8):
                    nc.vector.memset(qt_z0[64:128, ct, :], 0.0)
                    nc.vector.memset(qt_z1[0:64, ct, :], 0.0)
                qtz.append((qt_z0, qt_z1))

            # ---- Q^T projection per 512-row chunk, dt-OUTER across all 8
            # PSUM banks: matmuls for contraction slice dt only need DMA
            # slice dt of xt/wlo, so compute starts ~2MB earlier. Written
            # into two zero-padded slabs (z0: odd-head partitions zeroed,
            # z1: even) so score matmuls contract over the full 128
            # partitions (FWL stays on, no PE mode switches). ----
            Ident = mybir.ActivationFunctionType.Identity
            for rc in range(2):
                wsel = wlo_sb if rc == 0 else whi_sb
                bsel = blo_sb if rc == 0 else bhi_sb
                qt_z0, qt_z1 = qtz[rc]
                # 4-bank half-passes, double-buffered: pass N+1's matmuls
                # overlap pass N's PSUM->SBUF evictions. The z0 eviction
                # rides the otherwise-idle scalar engine (Identity
                # activation with per-partition bias), z1 the vector
                # engine, so neither engine serializes the PE.
                for cg in range(2):
                    # alternating tags so each pass's evictions depend only
                    # on that pass's matmuls (tag-level RAW tracking), and
                    # pass N+1's matmuls only WAR against pass N-1's evicts.
                    pq = qpsum.tile([128, 4, 512], F32,
                                    tag=f"pq{(rc * 2 + cg) % 2}", name="pq",
                                    bufs=1)
                    for dt in range(8):
                        for ci in range(4):
                            ct = cg * 4 + ci
                            nc.tensor.matmul(
                                pq[:, ci, :],
                                wsel[:, dt, ct * 128:(ct + 1) * 128],
                                xt_sb[:, dt, rc * 512:(rc + 1) * 512],
                                start=(dt == 0), stop=(dt == 7))
                    for ci in range(4):
                        ct = cg * 4 + ci
                        nc.scalar.activation(
                            out=qt_z0[0:64, ct, :], in_=pq[0:64, ci, :],
                            func=Ident, bias=bsel[0:64, ct:ct + 1])
                        nc.vector.tensor_scalar_add(
                            qt_z1[64:128, ct, :], pq[64:128, ci, :],
                            bsel[64:128, ct:ct + 1])
            qts = qtz
            projstack.close()

            # ---- attention, software-pipelined over the 8 head-pair
            # groups (ct): scores+exp run one group ahead of AV, two ahead
            # of the normalization. The K and V projections for pair ct+1
            # run inside iteration ct (PE work that overlaps the scalar
            # Exp). Both heads of a group share one praw2 slab, one
            # reciprocal, and one [128, 1024] normalize multiply. The
            # 1/sums broadcast is a PE ones-outer-product written into
            # partitions 0-127 of the group's SECOND pav tile (its rows
            # were already evacuated), so no PSUM banks are added and the
            # DVE multiply reads it with mixed partition bases. ----
            attnstack = ExitStack()
            psum = attnstack.enter_context(
                tc.tile_pool(name="psum", bufs=2, space="PSUM"))
            attnpool = attnstack.enter_context(tc.tile_pool(name="attnpool", bufs=1))
            ptpool = attnstack.enter_context(tc.tile_pool(name="ptpool", bufs=8))
            tmppool = attnstack.enter_context(tc.tile_pool(name="tmppool", bufs=4))
            rcppool = attnstack.enter_context(tc.tile_pool(name="rcppool", bufs=3))
            outpool = attnstack.enter_context(tc.tile_pool(name="outpool", bufs=4))
            attnT = attnpool.tile([128, 8, RPC], BF16, name="attnT")

            def kvproj(j):
                # K^T and V projections for head pair j, one PSUM "work"
                # alloc (bank 0 = K^T pair slab, bank 1 = V [a,4x128]).
                kv = psum.tile([128, 2, 512], F32, tag="work", name="kv",
                               bufs=2)
                for dt in range(8):
                    nc.tensor.matmul(
                        kv[:, 0, :], wk_sb[:, j, dt, :],
                        at_sb[:, dt, :], start=(dt == 0), stop=(dt == 7))
                kvv = kv[:, 1, :].rearrange("p (a c) -> p a c", a=4)
                for ach in range(4):
                    for dt in range(8):
                        nc.tensor.matmul(
                            kvv[:, ach, :],
                            at_sb[:, dt, ach * 128:(ach + 1) * 128],
                            wv_sb[:, j, dt, :],
                            start=(dt == 0), stop=(dt == 7))
                nc.vector.tensor_copy(kt_sb[:, j, :], kv[:, 0, :])
                vsrc = kv[:, 1, :].rearrange("p (a h d) -> p a h d", a=4, h=2)
                nc.vector.tensor_copy(vaug[:, :, 2 * j:2 * j + 2, 0:DH], vsrc)

            def stage_scores(ct):
                st = {"pts": []}
                for par in range(2):
                    for rc in range(2):
                        qt_sb = qts[rc][par]
                        pt = ptpool.tile([128, 4, 512], BF16, tag="pt",
                                         name="pt")
                        for half in range(2):
                            s2 = psum.tile([128, 2, 512], F32, tag="s",
                                           name="s2", bufs=2)
                            for k in range(2):
                                ach = 2 * half + k
                                nc.tensor.matmul(
                                    s2[:, k, :],
                                    kt_sb[:, ct, ach * 128:(ach + 1) * 128],
                                    qt_sb[:, ct, :],
                                    start=True, stop=True)
                            nc.scalar.activation(
                                out=pt[:, 2 * half:2 * half + 2, :], in_=s2,
                                func=Exp, scale=SCALE)
                        st["pts"].append(pt)
                return st

            def stage_av(ct, par, st):
                h = 2 * ct + par
                pav = psum.tile([128, 2, 512], F32, tag="work", name="pav",
                                bufs=2)
                for rc in range(2):
                    pt = st["pts"][par * 2 + rc]
                    for ach in range(4):
                        nc.tensor.matmul(
                            pav[0:DH + 1, rc, :], vaug[:, ach, h, :],
                            pt[:, ach, :], start=(ach == 0), stop=(ach == 3))
                if par == 0:
                    st["praw2"] = tmppool.tile([128, 2, 512], BF16,
                                               tag="praw", name="praw2")
                    # sums gathered to partition bases {0,64} of one tile
                    # so the reciprocal+cast run lane-parallel
                    st["sums4"] = rcppool.tile([128, 2, 512], F32,
                                               tag="sums", name="sums4")
                nc.vector.tensor_copy(st["praw2"][par * 64:par * 64 + DH, :, :],
                                      pav[0:DH, :, :])
                row = par * 64
                nc.vector.tensor_copy(st["sums4"][row:row + 1, :, :],
                                      pav[DH:DH + 1, :, :])
                st[f"pav{par}"] = pav

            def stage_recip(ct, st):
                rcp4 = rcppool.tile([128, 2, 512], F32, tag="rcp",
                                    name="rcp4")
                nc.vector.reciprocal_approx_fast(rcp4, st["sums4"])
                rcpbf = rcppool.tile([128, 2, 512], BF16, tag="rcpbf",
                                     name="rcpbf")
                nc.vector.tensor_copy(rcpbf, rcp4)
                st["rcpbf"] = rcpbf

            def stage_norm(ct, st):
                pav1 = st["pav1"]
                for par in range(2):
                    row = par * 64
                    for rcn in range(2):
                        nc.tensor.matmul(
                            pav1[par * 64:(par + 1) * 64, rcn, :],
                            ones_bf[row:row + 1, :],
                            st["rcpbf"][row:row + 1, rcn, :],
                            start=True, stop=True)
                dst = attnT[:, ct, :].rearrange("p (b r) -> p b r", b=2)
                nc.vector.tensor_mul(dst, st["praw2"], pav1)

            # O-proj partials for the first two row-tiles are emitted
            # inside the pipeline drain so the PE has work while the last
            # groups' normalization chains run. They live in "s"-tag PSUM
            # (free once the last scores are exp'd); [:, nh, :] holds the
            # nh-th 512-col half.
            pouts_head = {}

            def oproj_partial(cts, rtis):
                for rti in rtis:
                    if rti not in pouts_head:
                        pouts_head[rti] = psum.tile(
                            [128, 2, 512], F32, tag="s", name="pout2",
                            bufs=2)
                    pout2 = pouts_head[rti]
                    for nh in range(2):
                        for ct2 in cts:
                            nc.tensor.matmul(
                                pout2[:, nh, :],
                                attnT[:, ct2, rti * 128:(rti + 1) * 128],
                                wo_sb[:, ct2, nh * 512:(nh + 1) * 512],
                                start=(ct2 == 0), stop=False)

            kvproj(0)
            sts = {}
            for i in range(10):
                if i < 8:
                    sts[i] = stage_scores(i)
                if i + 1 < 8:
                    kvproj(i + 1)
                if i == 8:
                    oproj_partial(range(6), [0, 1])
                if i == 9:
                    oproj_partial([6], [0, 1])
                if 2 <= i <= 9:
                    stage_recip(i - 2, sts[i - 2])
                    stage_norm(i - 2, sts[i - 2])
                if 1 <= i <= 8:
                    stage_av(i - 1, 0, sts[i - 1])
                    stage_av(i - 1, 1, sts[i - 1])

            # ---- output projection ----
            for rti in range(8):
                for nh in range(2):
                    if rti <= 1:
                        pout = pouts_head[rti][:, nh, :]
                        nc.tensor.matmul(
                            pout, attnT[:, 7, rti * 128:(rti + 1) * 128],
                            wo_sb[:, 7, nh * 512:(nh + 1) * 512],
                            start=False, stop=True)
                    else:
                        pout = psum.tile([128, 512], F32, tag="work",
                                         name="pout")
                        for ct2 in range(8):
                            nc.tensor.matmul(
                                pout, attnT[:, ct2, rti * 128:(rti + 1) * 128],
                                wo_sb[:, ct2, nh * 512:(nh + 1) * 512],
                                start=(ct2 == 0), stop=(ct2 == 7))
                    out_t = outpool.tile([128, 512], BF16, tag="out",
                                         name="out_t")
                    nc.vector.tensor_add(out_t, pout,
                                         bo_bc[:, nh * 512:(nh + 1) * 512])
                    nc.sync.dma_start(
                        out=out.ap()[rti * 128:(rti + 1) * 128,
                                     nh * 512:(nh + 1) * 512],
                        in_=out_t)
            attnstack.close()

    nc.compile()
    return nc


def _swz(a):
    """[1024, cols] -> [128, 8, cols] with row r -> (r % 128, r // 128)."""
    return np.ascontiguousarray(
        a.reshape(8, 128, -1).transpose(1, 0, 2))


def _make_in_maps(x, Wq, bq, Wk, bk, Wv, bv, Wqt, bqt, Wo, bo):
    x = np.asarray(x, dtype=np.float32)
    bf = ml_dtypes.bfloat16

    wq_b = np.ascontiguousarray(np.asarray(Wq, np.float32).astype(bf))
    wqt_b = np.ascontiguousarray(np.asarray(Wqt, np.float32).astype(bf))
    wk_b = np.ascontiguousarray(np.asarray(Wk, np.float32).astype(bf))
    wv_b = np.ascontiguousarray(np.asarray(Wv, np.float32).astype(bf))
    wo_b = np.ascontiguousarray(np.asarray(Wo, np.float32).astype(bf))
    colmaj = lambda v: np.ascontiguousarray(
        np.asarray(v, np.float32).reshape(8, 128).T)
    bq, bqt = map(colmaj, (bq, bqt))
    # bv folded through Wo into the output bias (softmax rows sum to 1);
    # bk dropped (constant score shift per row, softmax-invariant).
    bo_eff = (np.asarray(bo, np.float32)
              + np.asarray(bv, np.float32) @ np.asarray(Wo, np.float32))
    bo_eff = np.ascontiguousarray(np.broadcast_to(bo_eff, (128, D)))

    wq_sw, wqt_sw = _swz(wq_b), _swz(wqt_b)
    pairmaj = lambda w: np.ascontiguousarray(
        _swz(w).reshape(128, 8, 8, 128).transpose(0, 2, 1, 3))
    wk_sw, wv_sw, wo_sw = pairmaj(wk_b), pairmaj(wv_b), _swz(wo_b)
    at_sw = [_swz(x[b, :A, :].T.astype(bf)) for b in range(B)]
    in_maps = []
    for c in range(NCORES):
        b, q = divmod(c, 4)
        rows = x[b, q * RPC:(q + 1) * RPC, :]
        in_maps.append({
            "xt": _swz(rows.T.astype(bf)),
            "at": at_sw[b],
            "wlo": wq_sw if q == 0 else wqt_sw,
            "whi": wqt_sw,
            "wk": wk_sw, "wv": wv_sw, "wo": wo_sw,
            "blo": bq if q == 0 else bqt, "bhi": bqt,
            "bo": bo_eff,
        })
    return in_maps


def kernel(x, Wq, bq, Wk, bk, Wv, bv, Wqt, bqt, Wo, bo, num_anchor_tokens):
    assert int(num_anchor_tokens) == A
    if "nc" not in _CACHE:
        _CACHE["nc"] = _build()
    nc = _CACHE["nc"]

    in_maps = _make_in_maps(x, Wq, bq, Wk, bk, Wv, bv, Wqt, bqt, Wo, bo)
    res = bass_utils.run_bass_kernel_spmd(
        nc, in_maps, core_ids=list(range(NCORES)))
    out = np.empty((B, S, D), np.float32)
    for c in range(NCORES):
        b, q = divmod(c, 4)
        out[b, q * RPC:(q + 1) * RPC, :] = res.results[c]["out"].astype(
            np.float32)
    return out


# revision 15
# speedup vs baseline: 1.0741x; 1.0035x over previous
"""AnchorAttention distributed Bass kernel for 8 TRN2 NeuronCores.

Reference computation (B=2, S=4096, D=1024, H=16, Dh=64, A=512):
  anchors = x[:, :A];  queries = x[:, A:]
  anchor_q/k/v = split_heads(anchors @ Wq/Wk/Wv + b)
  query_q      = split_heads(queries @ Wqt + bqt)
  combined_q   = concat([anchor_q, query_q], axis=2)       # [B,H,S,Dh]
  out  = softmax(combined_q @ anchor_k^T / sqrt(Dh)) @ anchor_v
  out  = merge_heads(out) @ Wo + bo

Sharding: the B*S = 8192 token rows are split into 8 chunks of 1024 rows
(core c -> batch c//4, rows (c%4)*1024). Each core duplicates its
batch's anchor K/V projections, computes Q for its own rows, attention
over the 512 anchors for all 16 heads, and the output projection for its
rows. The output is a pure concatenation: no collectives.

Bias algebra (host-side):
  * bk is dropped entirely: adding bk to K shifts every anchor's score
    for a given row by the same amount, and softmax is shift-invariant.
  * bv is folded into bo: softmax rows sum to 1, so attn @ (V + bv) =
    attn @ V + bv, and (out + bv) @ Wo + bo = out @ Wo + (bv @ Wo + bo).

Layout: everything is kept transposed ([feature, row]) so each matmul
contracts over the partition dim with zero on-chip transposes; the final
output projection naturally lands un-transposed [row, feature] for DMA
out. Host pre-transposes/pre-casts inputs to bf16 (compute dtype; f32
accumulation in PSUM). Softmax row-sums come free via an extra all-ones
column appended to V; no max-subtraction is needed (scores are ~N(0,1),
exp stays in a tiny range; softmax is shift-invariant so results match).

Schedule (the big difference vs the naive phase ordering): input DMAs are
issued in consumption order with xt/wlo split per contraction slice, and
the Q projection runs dt-outer over all 8 PSUM banks so the PE starts as
soon as the first 512KB lands instead of waiting for whole slabs. The
K and V projections are folded INTO the attention pair pipeline (one
head-pair's worth per iteration, one iteration ahead), so their PE work
overlaps the scalar-engine Exp activations, which are the per-pair
bottleneck otherwise.

Heads are packed two per 128-partition tile (head h -> column-tile h//2,
partitions (h%2)*64 ..). Attention is software-pipelined over the 8 head
pairs: scores+exp run one pair ahead of AV, two ahead of normalization.
AV appends an all-ones V column so softmax sums fall out of the matmul;
1/sums (fast DVE reciprocal, lane-parallel at partition bases 0/64) is
partition-broadcast by a PE ones-outer-product written into already-
evacuated rows of the pair's second PSUM tile, and a single mixed-
partition-base DVE multiply writes the normalized attn^T slab.
"""

from contextlib import ExitStack

import numpy as np
import ml_dtypes

import concourse.bass as bass
import concourse.tile as tile
from concourse import bacc, mybir
from concourse import bass_utils

BF16 = mybir.dt.bfloat16
F32 = mybir.dt.float32
B, S, D = 2, 4096, 1024
H, DH = 16, 64
A = 512                  # num_anchor_tokens (asserted at runtime)
RPC = 1024               # rows per core
NCORES = 8
SCALE = 1.0 / np.sqrt(float(DH))

_CACHE = {}


def _build():
    """Build + compile the per-core Bass graph (identical on all cores)."""
    nc = bacc.Bacc("TRN2", target_bir_lowering=False, debug=False)

    xt = nc.dram_tensor("xt", [128, 8, RPC], BF16, kind="ExternalInput")   # rows^T swizzled
    at = nc.dram_tensor("at", [128, 8, A], BF16, kind="ExternalInput")     # anchors^T swizzled
    wlo = nc.dram_tensor("wlo", [128, 8, D], BF16, kind="ExternalInput")   # Q weight rows 0-511
    whi = nc.dram_tensor("whi", [128, 8, D], BF16, kind="ExternalInput")   # Q weight rows 512-1023
    wk = nc.dram_tensor("wk", [128, 8, 8, 128], BF16, kind="ExternalInput")  # pair-major
    wv = nc.dram_tensor("wv", [128, 8, 8, 128], BF16, kind="ExternalInput")  # pair-major
    wo = nc.dram_tensor("wo", [128, 8, D], BF16, kind="ExternalInput")
    blo = nc.dram_tensor("blo", [128, 8], F32, kind="ExternalInput")
    bhi = nc.dram_tensor("bhi", [128, 8], F32, kind="ExternalInput")
    bo = nc.dram_tensor("bo", [128, D], F32, kind="ExternalInput")   # pre-broadcast bo + bv@Wo

    out = nc.dram_tensor("out", [RPC, D], BF16, kind="ExternalOutput")

    Exp = mybir.ActivationFunctionType.Exp

    with tile.TileContext(nc) as tc:
        with tc.tile_pool(name="wpool", bufs=1) as wpool, \
             tc.tile_pool(name="cpool", bufs=1) as cpool, \
             tc.tile_pool(name="kvpool", bufs=1) as kvpool, \
             tc.tile_pool(name="qtpool", bufs=2) as qtpool:
            # x + Q weights live only through the Q projection; their pools
            # (and the 8-bank Q PSUM pool) close before the attention pools
            # open so the attention working set reuses their space.
            projstack = ExitStack()
            wqpool = projstack.enter_context(tc.tile_pool(name="wqpool", bufs=1))
            xpool = projstack.enter_context(tc.tile_pool(name="xpool", bufs=1))
            qpsum = projstack.enter_context(
                tc.tile_pool(name="qpsum", bufs=1, space="PSUM"))

            # ---- DMA issue order == consumption order. xt/wlo are split
            # per dt slice so the dt-outer Q projection starts on slice 0
            # while the rest stream in; everything later is whole-slab. ----
            blo_sb = cpool.tile([128, 8], F32, name="blo_sb")
            nc.sync.dma_start(out=blo_sb, in_=blo.ap())
            bhi_sb = cpool.tile([128, 8], F32, name="bhi_sb")
            nc.sync.dma_start(out=bhi_sb, in_=bhi.ap())

            xt_sb = xpool.tile([128, 8, RPC], BF16, name="xt_sb")
            wlo_sb = wqpool.tile([128, 8, D], BF16, name="wlo_sb")
            whi_sb = wqpool.tile([128, 8, D], BF16, name="whi_sb")
            for k in range(8):
                nc.sync.dma_start(out=xt_sb[:, k:k + 1, :],
                                  in_=xt.ap()[:, k:k + 1, :])
                nc.sync.dma_start(out=wlo_sb[:, k:k + 1, :],
                                  in_=wlo.ap()[:, k:k + 1, :])
            for k in range(8):
                nc.sync.dma_start(out=whi_sb[:, k:k + 1, :],
                                  in_=whi.ap()[:, k:k + 1, :])

            at_sb = wpool.tile([128, 8, A], BF16, name="at_sb")
            nc.sync.dma_start(out=at_sb, in_=at.ap())
            # K/V weights stream in pair-major slices: pair j's attention
            # iteration only needs slice j, so attention starts ~20us
            # earlier than waiting for whole slabs at ~185 GB/s.
            wk_sb = wpool.tile([128, 8, 8, 128], BF16, name="wk_sb")
            wv_sb = wpool.tile([128, 8, 8, 128], BF16, name="wv_sb")
            for j in range(8):
                nc.sync.dma_start(out=wk_sb[:, j:j + 1, :, :],
                                  in_=wk.ap()[:, j:j + 1, :, :])
                nc.sync.dma_start(out=wv_sb[:, j:j + 1, :, :],
                                  in_=wv.ap()[:, j:j + 1, :, :])
            wo_sb = wpool.tile([128, 8, D], BF16, name="wo_sb")
            nc.sync.dma_start(out=wo_sb, in_=wo.ap())
            bo_bc = cpool.tile([128, D], F32, name="bo_bc")
            nc.sync.dma_start(out=bo_bc, in_=bo.ap())

            ones_bf = cpool.tile([128, DH], BF16, name="ones_bf")
            nc.vector.memset(ones_bf, 1.0)

            # V slab: [128(a%128), ach, head, 65]; cols 0-63 = V head slice,
            # col 64 = ones (supplies softmax row-sums during AV).
            vaug = kvpool.tile([128, 4, H, DH + 1], BF16, name="vaug")
            nc.vector.memset(vaug, 1.0)
            kt_sb = kvpool.tile([128, 8, A], BF16, name="kt_sb")

            qtz = []
            for rc in range(2):
                qt_z0 = qtpool.tile([128, 8, 512], BF16, tag=f"qt0_{rc}",
                                    name=f"qt_z0_{rc}", bufs=1)
                qt_z1 = qtpool.tile([128, 8, 512], BF16, tag=f"qt1_{rc}",
                                    name=f"qt_z1_{rc}", bufs=1)
                for ct in range(8):
                    nc.scalar.memzero(qt_z0[64:128, ct, :])
                    nc.vector.memset(qt_z1[0:64, ct, :], 0.0)
                qtz.append((qt_z0, qt_z1))

            # ---- Q^T projection per 512-row chunk, dt-OUTER across all 8
            # PSUM banks: matmuls for contraction slice dt only need DMA
            # slice dt of xt/wlo, so compute starts ~2MB earlier. Written
            # into two zero-padded slabs (z0: odd-head partitions zeroed,
            # z1: even) so score matmuls contract over the full 128
            # partitions (FWL stays on, no PE mode switches). ----
            Ident = mybir.ActivationFunctionType.Identity
            for rc in range(2):
                wsel = wlo_sb if rc == 0 else whi_sb
                bsel = blo_sb if rc == 0 else bhi_sb
                qt_z0, qt_z1 = qtz[rc]
                # 4-bank half-passes, double-buffered: pass N+1's matmuls
                # overlap pass N's PSUM->SBUF evictions. The z0 eviction
                # rides the otherwise-idle scalar engine (Identity
                # activation with per-partition bias), z1 the vector
                # engine, so neither engine serializes the PE.
                for cg in range(2):
                    # alternating tags so each pass's evictions depend only
                    # on that pass's matmuls (tag-level RAW tracking), and
                    # pass N+1's matmuls only WAR against pass N-1's evicts.
                    pq = qpsum.tile([128, 4, 512], F32,
                                    tag=f"pq{(rc * 2 + cg) % 2}", name="pq",
                                    bufs=1)
                    for dt in range(7):
                        for ci in range(4):
                            ct = cg * 4 + ci
                            nc.tensor.matmul(
                                pq[:, ci, :],
                                wsel[:, dt, ct * 128:(ct + 1) * 128],
                                xt_sb[:, dt, rc * 512:(rc + 1) * 512],
                                start=(dt == 0), stop=False)
                    # final contraction slice + eviction per bank, emitted
                    # bank-by-bank so each eviction's dependency covers only
                    # the matmuls before it and the chain hides under the
                    # rest of the round / the next pass.
                    for ci in range(4):
                        ct = cg * 4 + ci
                        nc.tensor.matmul(
                            pq[:, ci, :],
                            wsel[:, 7, ct * 128:(ct + 1) * 128],
                            xt_sb[:, 7, rc * 512:(rc + 1) * 512],
                            start=False, stop=True)
                        nc.scalar.activation(
                            out=qt_z0[0:64, ct, :], in_=pq[0:64, ci, :],
                            func=Ident, bias=bsel[0:64, ct:ct + 1])
                        nc.vector.tensor_scalar_add(
                            qt_z1[64:128, ct, :], pq[64:128, ci, :],
                            bsel[64:128, ct:ct + 1])
            qts = qtz
            projstack.close()

            # ---- attention, software-pipelined over the 8 head-pair
            # groups (ct): scores+exp run one group ahead of AV, two ahead
            # of the normalization. The K and V projections for pair ct+1
            # run inside iteration ct (PE work that overlaps the scalar
            # Exp). Both heads of a group share one praw2 slab, one
            # reciprocal, and one [128, 1024] normalize multiply. The
            # 1/sums broadcast is a PE ones-outer-product written into
            # partitions 0-127 of the group's SECOND pav tile (its rows
            # were already evacuated), so no PSUM banks are added and the
            # DVE multiply reads it with mixed partition bases. ----
            attnstack = ExitStack()
            psum = attnstack.enter_context(
                tc.tile_pool(name="psum", bufs=2, space="PSUM"))
            attnpool = attnstack.enter_context(tc.tile_pool(name="attnpool", bufs=1))
            ptpool = attnstack.enter_context(tc.tile_pool(name="ptpool", bufs=8))
            tmppool = attnstack.enter_context(tc.tile_pool(name="tmppool", bufs=4))
            rcppool = attnstack.enter_context(tc.tile_pool(name="rcppool", bufs=3))
            outpool = attnstack.enter_context(tc.tile_pool(name="outpool", bufs=4))
            attnT = attnpool.tile([128, 8, RPC], BF16, name="attnT")

            def kv_units(j):
                # K^T and V projections for head pair j, returned as small
                # emission units that get interleaved between score units:
                # the in-order PE then always has independent matmuls in
                # flight while a scores matmul waits on Exp buffer reuse.
                kv = psum.tile([128, 2, 512], F32, tag="work", name="kv",
                               bufs=2)
                kvv = kv[:, 1, :].rearrange("p (a c) -> p a c", a=4)

                def KV0(h0, last=False):
                    for dt in range(h0, h0 + 4):
                        nc.tensor.matmul(
                            kv[:, 0, :], wk_sb[:, j, dt, :],
                            at_sb[:, dt, :], start=(dt == 0), stop=(dt == 7))
                        nc.tensor.matmul(
                            kvv[:, 0, :], at_sb[:, dt, 0:128],
                            wv_sb[:, j, dt, :],
                            start=(dt == 0), stop=(dt == 7))
                    if last:
                        nc.vector.tensor_copy(kt_sb[:, j, :], kv[:, 0, :])

                def V(achs, last=False):
                    for ach in achs:
                        for dt in range(8):
                            nc.tensor.matmul(
                                kvv[:, ach, :],
                                at_sb[:, dt, ach * 128:(ach + 1) * 128],
                                wv_sb[:, j, dt, :],
                                start=(dt == 0), stop=(dt == 7))
                    if last:
                        vsrc = kv[:, 1, :].rearrange("p (a h d) -> p a h d",
                                                     a=4, h=2)
                        nc.vector.tensor_copy(
                            vaug[:, :, 2 * j:2 * j + 2, 0:DH], vsrc)

                return [lambda: KV0(0), lambda: KV0(4, True),
                        lambda: V((1, 2)), lambda: V((3,), True)]

            def score_unit(ct, st, par, rc, half):
                qt_sb = qts[rc][par]
                pt = st["pts"][par * 2 + rc]
                s2 = psum.tile([128, 2, 512], F32, tag="s", name="s2",
                               bufs=2)
                for k in range(2):
                    ach = 2 * half + k
                    nc.tensor.matmul(
                        s2[:, k, :],
                        kt_sb[:, ct, ach * 128:(ach + 1) * 128],
                        qt_sb[:, ct, :],
                        start=True, stop=True)
                nc.scalar.activation(
                    out=pt[:, 2 * half:2 * half + 2, :], in_=s2,
                    func=Exp, scale=SCALE)

            def stage_av(ct, par, st):
                h = 2 * ct + par
                pav = psum.tile([128, 2, 512], F32, tag="work", name="pav",
                                bufs=2)
                for ach in range(4):
                    for rc in range(2):
                        pt = st["pts"][par * 2 + rc]
                        nc.tensor.matmul(
                            pav[0:DH + 1, rc, :], vaug[:, ach, h, :],
                            pt[:, ach, :], start=(ach == 0), stop=(ach == 3))
                if par == 0:
                    st["praw2"] = tmppool.tile([128, 2, 512], BF16,
                                               tag="praw", name="praw2")
                    # sums gathered to partition bases {0,64} of one tile
                    # so the reciprocal+cast run lane-parallel
                    st["sums4"] = rcppool.tile([128, 2, 512], F32,
                                               tag="sums", name="sums4")
                nc.vector.tensor_copy(st["praw2"][par * 64:par * 64 + DH, :, :],
                                      pav[0:DH, :, :])
                row = par * 64
                nc.vector.tensor_copy(st["sums4"][row:row + 1, :, :],
                                      pav[DH:DH + 1, :, :])
                st[f"pav{par}"] = pav

            def stage_recipnorm(ct, st):
                rcp4 = rcppool.tile([128, 2, 512], F32, tag="rcp",
                                    name="rcp4")
                nc.vector.reciprocal_approx_fast(rcp4, st["sums4"])
                rcpbf = rcppool.tile([128, 2, 512], BF16, tag="rcpbf",
                                     name="rcpbf")
                nc.vector.tensor_copy(rcpbf, rcp4)
                pav1 = st["pav1"]
                for par in range(2):
                    row = par * 64
                    for rcn in range(2):
                        nc.tensor.matmul(
                            pav1[par * 64:(par + 1) * 64, rcn, :],
                            ones_bf[row:row + 1, :],
                            rcpbf[row:row + 1, rcn, :],
                            start=True, stop=True)
                dst = attnT[:, ct, :].rearrange("p (b r) -> p b r", b=2)
                nc.vector.tensor_mul(dst, st["praw2"], pav1)

            # O-proj partials for the first two row-tiles are emitted
            # inside the pipeline drain so the PE has work while the last
            # groups' normalization chains run. They live in "s"-tag PSUM
            # (free once the last scores are exp'd); [:, nh, :] holds the
            # nh-th 512-col half.
            pouts_head = {}

            def oproj_partial(cts, rtis):
                for rti in rtis:
                    if rti not in pouts_head:
                        pouts_head[rti] = psum.tile(
                            [128, 2, 512], F32, tag="s", name="pout2",
                            bufs=2)
                    pout2 = pouts_head[rti]
                    for ct2 in cts:
                        for nh in range(2):
                            nc.tensor.matmul(
                                pout2[:, nh, :],
                                attnT[:, ct2, rti * 128:(rti + 1) * 128],
                                wo_sb[:, ct2, nh * 512:(nh + 1) * 512],
                                start=(ct2 == 0), stop=False)

            kv_units_0 = kv_units(0)
            for u in kv_units_0:
                u()
            sts = {}
            for i in range(10):
                fill = kv_units(i + 1) if i + 1 < 8 else []
                if i < 8:
                    st = sts[i] = {"pts": [
                        ptpool.tile([128, 4, 512], BF16, tag="pt", name="pt")
                        for _ in range(4)]}
                    # emission order: score units spaced by filler units so
                    # the PSUM "s" rotation (alloc n WARs exp(n-2)) never
                    # stalls the in-order PE.
                    score_unit(i, st, 0, 0, 0)
                    score_unit(i, st, 0, 0, 1)
                    fill[0]()
                    score_unit(i, st, 0, 1, 0)
                    fill[1]()
                    score_unit(i, st, 0, 1, 1)
                    fill[2]()
                    score_unit(i, st, 1, 0, 0)
                    fill[3]()
                    score_unit(i, st, 1, 0, 1)
                    if 2 <= i:
                        stage_recipnorm(i - 2, sts[i - 2])
                    score_unit(i, st, 1, 1, 0)
                    if 1 <= i:
                        stage_av(i - 1, 0, sts[i - 1])
                    score_unit(i, st, 1, 1, 1)
                    if 1 <= i:
                        stage_av(i - 1, 1, sts[i - 1])
                else:
                    if i == 8:
                        oproj_partial(range(6), [0, 1])
                    if i == 9:
                        oproj_partial([6], [0, 1])
                    stage_recipnorm(i - 2, sts[i - 2])
                    if i <= 8:
                        stage_av(i - 1, 0, sts[i - 1])
                        stage_av(i - 1, 1, sts[i - 1])

            # ---- output projection ----
            for rti in range(8):
                if rti <= 1:
                    pouts = [pouts_head[rti][:, nh, :] for nh in range(2)]
                    for nh in range(2):
                        nc.tensor.matmul(
                            pouts[nh], attnT[:, 7, rti * 128:(rti + 1) * 128],
                            wo_sb[:, 7, nh * 512:(nh + 1) * 512],
                            start=False, stop=True)
                else:
                    pouts = [psum.tile([128, 512], F32, tag="work",
                                       name="pout") for _ in range(2)]
                    for ct2 in range(8):
                        for nh in range(2):
                            nc.tensor.matmul(
                                pouts[nh],
                                attnT[:, ct2, rti * 128:(rti + 1) * 128],
                                wo_sb[:, ct2, nh * 512:(nh + 1) * 512],
                                start=(ct2 == 0), stop=(ct2 == 7))
                for nh in range(2):
                    out_t = outpool.tile([128, 512], BF16, tag="out",
                                         name="out_t")
                    nc.vector.tensor_add(out_t, pouts[nh],
                                         bo_bc[:, nh * 512:(nh + 1) * 512])
                    nc.sync.dma_start(
                        out=out.ap()[rti * 128:(rti + 1) * 128,
                                     nh * 512:(nh + 1) * 512],
                        in_=out_t)
            attnstack.close()

    nc.compile()
    return nc


def _swz(a):
    """[1024, cols] -> [128, 8, cols] with row r -> (r % 128, r // 128)."""
    return np.ascontiguousarray(
        a.reshape(8, 128, -1).transpose(1, 0, 2))


def _make_in_maps(x, Wq, bq, Wk, bk, Wv, bv, Wqt, bqt, Wo, bo):
    x = np.asarray(x, dtype=np.float32)
    bf = ml_dtypes.bfloat16

    wq_b = np.ascontiguousarray(np.asarray(Wq, np.float32).astype(bf))
    wqt_b = np.ascontiguousarray(np.asarray(Wqt, np.float32).astype(bf))
    wk_b = np.ascontiguousarray(np.asarray(Wk, np.float32).astype(bf))
    wv_b = np.ascontiguousarray(np.asarray(Wv, np.float32).astype(bf))
    wo_b = np.ascontiguousarray(np.asarray(Wo, np.float32).astype(bf))
    colmaj = lambda v: np.ascontiguousarray(
        np.asarray(v, np.float32).reshape(8, 128).T)
    bq, bqt = map(colmaj, (bq, bqt))
    # bv folded through Wo into the output bias (softmax rows sum to 1);
    # bk dropped (constant score shift per row, softmax-invariant).
    bo_eff = (np.asarray(bo, np.float32)
              + np.asarray(bv, np.float32) @ np.asarray(Wo, np.float32))
    bo_eff = np.ascontiguousarray(np.broadcast_to(bo_eff, (128, D)))

    wq_sw, wqt_sw = _swz(wq_b), _swz(wqt_b)
    pairmaj = lambda w: np.ascontiguousarray(
        _swz(w).reshape(128, 8, 8, 128).transpose(0, 2, 1, 3))
    wk_sw, wv_sw, wo_sw = pairmaj(wk_b), pairmaj(wv_b), _swz(wo_b)
    at_sw = [_swz(x[b, :A, :].T.astype(bf)) for b in range(B)]
    in_maps = []
    for c in range(NCORES):
        b, q = divmod(c, 4)
        rows = x[b, q * RPC:(q + 1) * RPC, :]
        in_maps.append({
            "xt": _swz(rows.T.astype(bf)),
            "at": at_sw[b],
            "wlo": wq_sw if q == 0 else wqt_sw,
            "whi": wqt_sw,
            "wk": wk_sw, "wv": wv_sw, "wo": wo_sw,
            "blo": bq if q == 0 else bqt, "bhi": bqt,
            "bo": bo_eff,
        })
    return in_maps


def kernel(x, Wq, bq, Wk, bk, Wv, bv, Wqt, bqt, Wo, bo, num_anchor_tokens):
    assert int(num_anchor_tokens) == A
    if "nc" not in _CACHE:
        _CACHE["nc"] = _build()
    nc = _CACHE["nc"]

    in_maps = _make_in_maps(x, Wq, bq, Wk, bk, Wv, bv, Wqt, bqt, Wo, bo)
    res = bass_utils.run_bass_kernel_spmd(
        nc, in_maps, core_ids=list(range(NCORES)))
    out = np.empty((B, S, D), np.float32)
    for c in range(NCORES):
        b, q = divmod(c, 4)
        out[b, q * RPC:(q + 1) * RPC, :] = res.results[c]["out"].astype(
            np.float32)
    return out


# revision 17
# speedup vs baseline: 1.0898x; 1.0146x over previous
"""AnchorAttention distributed Bass kernel for 8 TRN2 NeuronCores.

Reference computation (B=2, S=4096, D=1024, H=16, Dh=64, A=512):
  anchors = x[:, :A];  queries = x[:, A:]
  anchor_q/k/v = split_heads(anchors @ Wq/Wk/Wv + b)
  query_q      = split_heads(queries @ Wqt + bqt)
  combined_q   = concat([anchor_q, query_q], axis=2)       # [B,H,S,Dh]
  out  = softmax(combined_q @ anchor_k^T / sqrt(Dh)) @ anchor_v
  out  = merge_heads(out) @ Wo + bo

Sharding: the B*S = 8192 token rows are split into 8 chunks of 1024 rows
(core c -> batch c//4, rows (c%4)*1024). Each core duplicates its
batch's anchor K/V projections, computes Q for its own rows, attention
over the 512 anchors for all 16 heads, and the output projection for its
rows. The output is a pure concatenation: no collectives.

Bias algebra (host-side):
  * bk is dropped entirely: adding bk to K shifts every anchor's score
    for a given row by the same amount, and softmax is shift-invariant.
  * bv is folded into bo: softmax rows sum to 1, so attn @ (V + bv) =
    attn @ V + bv, and (out + bv) @ Wo + bo = out @ Wo + (bv @ Wo + bo).

Layout: everything is kept transposed ([feature, row]) so each matmul
contracts over the partition dim with zero on-chip transposes; the final
output projection naturally lands un-transposed [row, feature] for DMA
out. Host pre-transposes/pre-casts inputs to bf16 (compute dtype; f32
accumulation in PSUM). Softmax row-sums come free via an extra all-ones
column appended to V; no max-subtraction is needed (scores are ~N(0,1),
exp stays in a tiny range; softmax is shift-invariant so results match).

Schedule (the big difference vs the naive phase ordering): input DMAs are
issued in consumption order with xt/wlo split per contraction slice, and
the Q projection runs dt-outer over all 8 PSUM banks so the PE starts as
soon as the first 512KB lands instead of waiting for whole slabs. The
K and V projections are folded INTO the attention pair pipeline (one
head-pair's worth per iteration, one iteration ahead), so their PE work
overlaps the scalar-engine Exp activations, which are the per-pair
bottleneck otherwise.

Heads are packed two per 128-partition tile (head h -> column-tile h//2,
partitions (h%2)*64 ..). Attention is software-pipelined over the 8 head
pairs: scores+exp run one pair ahead of AV, two ahead of normalization.
AV appends an all-ones V column so softmax sums fall out of the matmul;
1/sums (fast DVE reciprocal, lane-parallel at partition bases 0/64) is
partition-broadcast by a PE ones-outer-product written into already-
evacuated rows of the pair's second PSUM tile, and a single mixed-
partition-base DVE multiply writes the normalized attn^T slab.
"""

from contextlib import ExitStack

import numpy as np
import ml_dtypes

import concourse.bass as bass
import concourse.tile as tile
from concourse import bacc, mybir
from concourse import bass_utils

BF16 = mybir.dt.bfloat16
F32 = mybir.dt.float32
B, S, D = 2, 4096, 1024
H, DH = 16, 64
A = 512                  # num_anchor_tokens (asserted at runtime)
RPC = 1024               # rows per core
NCORES = 8
SCALE = 1.0 / np.sqrt(float(DH))

_CACHE = {}


def _build():
    """Build + compile the per-core Bass graph (identical on all cores)."""
    nc = bacc.Bacc("TRN2", target_bir_lowering=False, debug=False)

    xt = nc.dram_tensor("xt", [128, 8, RPC], BF16, kind="ExternalInput")   # rows^T swizzled
    at = nc.dram_tensor("at", [128, 8, A], BF16, kind="ExternalInput")     # anchors^T swizzled
    wlo = nc.dram_tensor("wlo", [128, 8, D], BF16, kind="ExternalInput")   # Q weight rows 0-511
    whi = nc.dram_tensor("whi", [128, 8, D], BF16, kind="ExternalInput")   # Q weight rows 512-1023
    wk = nc.dram_tensor("wk", [128, 8, 8, 128], BF16, kind="ExternalInput")  # pair-major
    wv = nc.dram_tensor("wv", [128, 8, 8, 128], BF16, kind="ExternalInput")  # pair-major
    wo = nc.dram_tensor("wo", [128, 8, D], BF16, kind="ExternalInput")
    blo = nc.dram_tensor("blo", [128, 8], F32, kind="ExternalInput")
    bhi = nc.dram_tensor("bhi", [128, 8], F32, kind="ExternalInput")
    bo = nc.dram_tensor("bo", [128, D], F32, kind="ExternalInput")   # pre-broadcast bo + bv@Wo

    out = nc.dram_tensor("out", [RPC, D], BF16, kind="ExternalOutput")

    Exp = mybir.ActivationFunctionType.Exp

    with tile.TileContext(nc) as tc:
        with tc.tile_pool(name="wpool", bufs=1) as wpool, \
             tc.tile_pool(name="cpool", bufs=1) as cpool, \
             tc.tile_pool(name="kvpool", bufs=1) as kvpool, \
             tc.tile_pool(name="qtpool", bufs=2) as qtpool:
            # x + Q weights live only through the Q projection; their pools
            # (and the 8-bank Q PSUM pool) close before the attention pools
            # open so the attention working set reuses their space.
            projstack = ExitStack()
            wqpool = projstack.enter_context(tc.tile_pool(name="wqpool", bufs=1))
            xpool = projstack.enter_context(tc.tile_pool(name="xpool", bufs=1))
            qpsum = projstack.enter_context(
                tc.tile_pool(name="qpsum", bufs=1, space="PSUM"))

            # ---- DMA issue order == consumption order. xt/wlo are split
            # per dt slice so the dt-outer Q projection starts on slice 0
            # while the rest stream in; everything later is whole-slab. ----
            blo_sb = cpool.tile([128, 8], F32, name="blo_sb")
            nc.sync.dma_start(out=blo_sb, in_=blo.ap())
            bhi_sb = cpool.tile([128, 8], F32, name="bhi_sb")
            nc.sync.dma_start(out=bhi_sb, in_=bhi.ap())

            xt_sb = xpool.tile([128, 8, RPC], BF16, name="xt_sb")
            wlo_sb = wqpool.tile([128, 8, D], BF16, name="wlo_sb")
            whi_sb = wqpool.tile([128, 8, D], BF16, name="whi_sb")
            for k in range(8):
                nc.sync.dma_start(out=xt_sb[:, k:k + 1, :],
                                  in_=xt.ap()[:, k:k + 1, :])
                nc.sync.dma_start(out=wlo_sb[:, k:k + 1, :],
                                  in_=wlo.ap()[:, k:k + 1, :])
            for k in range(8):
                nc.sync.dma_start(out=whi_sb[:, k:k + 1, :],
                                  in_=whi.ap()[:, k:k + 1, :])

            at_sb = wpool.tile([128, 8, A], BF16, name="at_sb")
            nc.sync.dma_start(out=at_sb, in_=at.ap())
            # K/V weights stream in pair-major slices: pair j's attention
            # iteration only needs slice j, so attention starts ~20us
            # earlier than waiting for whole slabs at ~185 GB/s.
            wk_sb = wpool.tile([128, 8, 8, 128], BF16, name="wk_sb")
            wv_sb = wpool.tile([128, 8, 8, 128], BF16, name="wv_sb")
            for j in range(8):
                nc.sync.dma_start(out=wk_sb[:, j:j + 1, :, :],
                                  in_=wk.ap()[:, j:j + 1, :, :])
                nc.sync.dma_start(out=wv_sb[:, j:j + 1, :, :],
                                  in_=wv.ap()[:, j:j + 1, :, :])
            wo_sb = wpool.tile([128, 8, D], BF16, name="wo_sb")
            nc.sync.dma_start(out=wo_sb, in_=wo.ap())
            bo_bc = cpool.tile([128, D], F32, name="bo_bc")
            nc.sync.dma_start(out=bo_bc, in_=bo.ap())

            ones_bf = cpool.tile([128, DH], BF16, name="ones_bf")
            nc.vector.memset(ones_bf, 1.0)

            # V slab: [128(a%128), ach, head, 65]; cols 0-63 = V head slice,
            # col 64 = ones (supplies softmax row-sums during AV).
            vaug = kvpool.tile([128, 4, H, DH + 1], BF16, name="vaug")
            nc.vector.memset(vaug, 1.0)
            kt_sb = kvpool.tile([128, 8, A], BF16, name="kt_sb")

            qtz = []
            for rc in range(2):
                qt_z0 = qtpool.tile([128, 8, 512], BF16, tag=f"qt0_{rc}",
                                    name=f"qt_z0_{rc}", bufs=1)
                qt_z1 = qtpool.tile([128, 8, 512], BF16, tag=f"qt1_{rc}",
                                    name=f"qt_z1_{rc}", bufs=1)
                for ct in range(8):
                    nc.scalar.memzero(qt_z0[64:128, ct, :])
                    nc.vector.memset(qt_z1[0:64, ct, :], 0.0)
                qtz.append((qt_z0, qt_z1))

            # ---- Q^T projection per 512-row chunk, dt-OUTER across all 8
            # PSUM banks: matmuls for contraction slice dt only need DMA
            # slice dt of xt/wlo, so compute starts ~2MB earlier. Written
            # into two zero-padded slabs (z0: odd-head partitions zeroed,
            # z1: even) so score matmuls contract over the full 128
            # partitions (FWL stays on, no PE mode switches). ----
            Ident = mybir.ActivationFunctionType.Identity
            for rc in range(2):
                wsel = wlo_sb if rc == 0 else whi_sb
                bsel = blo_sb if rc == 0 else bhi_sb
                qt_z0, qt_z1 = qtz[rc]
                # 4-bank half-passes, double-buffered: pass N+1's matmuls
                # overlap pass N's PSUM->SBUF evictions. The z0 eviction
                # rides the otherwise-idle scalar engine (Identity
                # activation with per-partition bias), z1 the vector
                # engine, so neither engine serializes the PE.
                for cg in range(4):
                    # alternating tags so each pass's evictions depend only
                    # on that pass's matmuls (tag-level RAW tracking), and
                    # pass N+1's matmuls only WAR against pass N-1's evicts.
                    pq = qpsum.tile([128, 2, 512], F32,
                                    tag=f"pq{cg % 2}", name="pq",
                                    bufs=1)
                    for dt in range(7):
                        for ci in range(2):
                            ct = cg * 2 + ci
                            nc.tensor.matmul(
                                pq[:, ci, :],
                                wsel[:, dt, ct * 128:(ct + 1) * 128],
                                xt_sb[:, dt, rc * 512:(rc + 1) * 512],
                                start=(dt == 0), stop=False)
                    # final contraction slice + eviction per bank, emitted
                    # bank-by-bank so each eviction's dependency covers only
                    # the matmuls before it and the chain hides under the
                    # rest of the round / the next pass.
                    for ci in range(2):
                        ct = cg * 2 + ci
                        nc.tensor.matmul(
                            pq[:, ci, :],
                            wsel[:, 7, ct * 128:(ct + 1) * 128],
                            xt_sb[:, 7, rc * 512:(rc + 1) * 512],
                            start=False, stop=True)
                        nc.scalar.activation(
                            out=qt_z0[0:64, ct, :], in_=pq[0:64, ci, :],
                            func=Ident, bias=bsel[0:64, ct:ct + 1])
                        nc.vector.tensor_scalar_add(
                            qt_z1[64:128, ct, :], pq[64:128, ci, :],
                            bsel[64:128, ct:ct + 1])
            qts = qtz
            projstack.close()

            # ---- attention, software-pipelined over the 8 head-pair
            # groups (ct): scores+exp run one group ahead of AV, two ahead
            # of the normalization. The K and V projections for pair ct+1
            # run inside iteration ct (PE work that overlaps the scalar
            # Exp). Both heads of a group share one praw2 slab, one
            # reciprocal, and one [128, 1024] normalize multiply. The
            # 1/sums broadcast is a PE ones-outer-product written into
            # partitions 0-127 of the group's SECOND pav tile (its rows
            # were already evacuated), so no PSUM banks are added and the
            # DVE multiply reads it with mixed partition bases. ----
            attnstack = ExitStack()
            psum = attnstack.enter_context(
                tc.tile_pool(name="psum", bufs=2, space="PSUM"))
            attnpool = attnstack.enter_context(tc.tile_pool(name="attnpool", bufs=1))
            ptpool = attnstack.enter_context(tc.tile_pool(name="ptpool", bufs=8))
            tmppool = attnstack.enter_context(tc.tile_pool(name="tmppool", bufs=4))
            rcppool = attnstack.enter_context(tc.tile_pool(name="rcppool", bufs=3))
            outpool = attnstack.enter_context(tc.tile_pool(name="outpool", bufs=4))
            attnT = attnpool.tile([128, 8, RPC], BF16, name="attnT")

            def kv_units(j):
                # K^T and V projections for head pair j, returned as small
                # emission units that get interleaved between score units:
                # the in-order PE then always has independent matmuls in
                # flight while a scores matmul waits on Exp buffer reuse.
                kv = psum.tile([128, 2, 512], F32, tag="work", name="kv",
                               bufs=2)
                kvv = kv[:, 1, :].rearrange("p (a c) -> p a c", a=4)

                def KV0(h0, last=False):
                    for dt in range(h0, h0 + 4):
                        nc.tensor.matmul(
                            kv[:, 0, :], wk_sb[:, j, dt, :],
                            at_sb[:, dt, :], start=(dt == 0), stop=(dt == 7))
                        nc.tensor.matmul(
                            kvv[:, 0, :], at_sb[:, dt, 0:128],
                            wv_sb[:, j, dt, :],
                            start=(dt == 0), stop=(dt == 7))
                    if last:
                        nc.vector.tensor_copy(kt_sb[:, j, :], kv[:, 0, :])

                def V(achs, last=False):
                    for ach in achs:
                        for dt in range(8):
                            nc.tensor.matmul(
                                kvv[:, ach, :],
                                at_sb[:, dt, ach * 128:(ach + 1) * 128],
                                wv_sb[:, j, dt, :],
                                start=(dt == 0), stop=(dt == 7))
                    if last:
                        vsrc = kv[:, 1, :].rearrange("p (a h d) -> p a h d",
                                                     a=4, h=2)
                        nc.vector.tensor_copy(
                            vaug[:, :, 2 * j:2 * j + 2, 0:DH], vsrc)

                return [lambda: KV0(0), lambda: KV0(4, True),
                        lambda: V((1, 2)), lambda: V((3,), True)]

            def score_unit(ct, st, par, rc, half):
                qt_sb = qts[rc][par]
                pt = st["pts"][par * 2 + rc]
                s2 = psum.tile([128, 2, 512], F32, tag="s", name="s2",
                               bufs=2)
                for k in range(2):
                    ach = 2 * half + k
                    nc.tensor.matmul(
                        s2[:, k, :],
                        kt_sb[:, ct, ach * 128:(ach + 1) * 128],
                        qt_sb[:, ct, :],
                        start=True, stop=True)
                nc.scalar.activation(
                    out=pt[:, 2 * half:2 * half + 2, :], in_=s2,
                    func=Exp, scale=SCALE)

            def stage_av(ct, par, st):
                h = 2 * ct + par
                pav = psum.tile([128, 2, 512], F32, tag="work", name="pav",
                                bufs=2)
                for ach in range(4):
                    for rc in range(2):
                        pt = st["pts"][par * 2 + rc]
                        nc.tensor.matmul(
                            pav[0:DH + 1, rc, :], vaug[:, ach, h, :],
                            pt[:, ach, :], start=(ach == 0), stop=(ach == 3))
                if par == 0:
                    st["praw2"] = tmppool.tile([128, 2, 512], BF16,
                                               tag="praw", name="praw2")
                    # sums gathered to partition bases {0,64} of one tile
                    # so the reciprocal+cast run lane-parallel
                    st["sums4"] = rcppool.tile([128, 2, 512], F32,
                                               tag="sums", name="sums4")
                nc.vector.tensor_copy(st["praw2"][par * 64:par * 64 + DH, :, :],
                                      pav[0:DH, :, :])
                row = par * 64
                nc.vector.tensor_copy(st["sums4"][row:row + 1, :, :],
                                      pav[DH:DH + 1, :, :])
                st[f"pav{par}"] = pav

            def stage_recipnorm(ct, st):
                rcp4 = rcppool.tile([128, 2, 512], F32, tag="rcp",
                                    name="rcp4")
                nc.vector.reciprocal_approx_fast(rcp4, st["sums4"])
                rcpbf = rcppool.tile([128, 2, 512], BF16, tag="rcpbf",
                                     name="rcpbf")
                nc.vector.tensor_copy(rcpbf, rcp4)
                pav1 = st["pav1"]
                for par in range(2):
                    row = par * 64
                    for rcn in range(2):
                        nc.tensor.matmul(
                            pav1[par * 64:(par + 1) * 64, rcn, :],
                            ones_bf[row:row + 1, :],
                            rcpbf[row:row + 1, rcn, :],
                            start=True, stop=True)
                dst = attnT[:, ct, :].rearrange("p (b r) -> p b r", b=2)
                nc.vector.tensor_mul(dst, st["praw2"], pav1)

            # O-proj partials for the first two row-tiles are emitted
            # inside the pipeline drain so the PE has work while the last
            # groups' normalization chains run. They live in "s"-tag PSUM
            # (free once the last scores are exp'd); [:, nh, :] holds the
            # nh-th 512-col half.
            pouts_head = {}

            def oproj_partial(cts, rtis):
                for rti in rtis:
                    if rti not in pouts_head:
                        pouts_head[rti] = psum.tile(
                            [128, 2, 512], F32, tag="s", name="pout2",
                            bufs=2)
                    pout2 = pouts_head[rti]
                    for ct2 in cts:
                        for nh in range(2):
                            nc.tensor.matmul(
                                pout2[:, nh, :],
                                attnT[:, ct2, rti * 128:(rti + 1) * 128],
                                wo_sb[:, ct2, nh * 512:(nh + 1) * 512],
                                start=(ct2 == 0), stop=False)

            kv_units_0 = kv_units(0)
            for u in kv_units_0:
                u()
            sts = {}
            for i in range(10):
                fill = kv_units(i + 1) if i + 1 < 8 else []
                if i < 8:
                    st = sts[i] = {"pts": [
                        ptpool.tile([128, 4, 512], BF16, tag="pt", name="pt")
                        for _ in range(4)]}
                    # emission order: score units spaced by filler units so
                    # the PSUM "s" rotation (alloc n WARs exp(n-2)) never
                    # stalls the in-order PE.
                    score_unit(i, st, 0, 0, 0)
                    score_unit(i, st, 0, 0, 1)
                    fill[0]()
                    score_unit(i, st, 0, 1, 0)
                    fill[1]()
                    score_unit(i, st, 0, 1, 1)
                    fill[2]()
                    score_unit(i, st, 1, 0, 0)
                    fill[3]()
                    score_unit(i, st, 1, 0, 1)
                    if 2 <= i:
                        stage_recipnorm(i - 2, sts[i - 2])
                    score_unit(i, st, 1, 1, 0)
                    if 1 <= i:
                        stage_av(i - 1, 0, sts[i - 1])
                    score_unit(i, st, 1, 1, 1)
                    if 1 <= i:
                        stage_av(i - 1, 1, sts[i - 1])
                else:
                    if i == 8:
                        oproj_partial(range(6), [0, 1])
                    if i == 9:
                        oproj_partial([6], [0, 1])
                    stage_recipnorm(i - 2, sts[i - 2])
                    if i <= 8:
                        stage_av(i - 1, 0, sts[i - 1])
                        stage_av(i - 1, 1, sts[i - 1])
                    if i == 9:
                        # rti 2/3 partials over pairs 0-6 fill the PE while
                        # the final normalization chain drains; their
                        # "work" allocs are safe now that norm(7) consumed
                        # the last pav tile.
                        for rti in (2, 3):
                            pouts_head[rti] = psum.tile(
                                [128, 2, 512], F32, tag="work",
                                name="pout2", bufs=2)
                            for ct2 in range(7):
                                for nh in range(2):
                                    nc.tensor.matmul(
                                        pouts_head[rti][:, nh, :],
                                        attnT[:, ct2,
                                              rti * 128:(rti + 1) * 128],
                                        wo_sb[:, ct2,
                                              nh * 512:(nh + 1) * 512],
                                        start=(ct2 == 0), stop=False)

            # ---- output projection ----
            for rti in range(8):
                if rti <= 3:
                    pouts = [pouts_head[rti][:, nh, :] for nh in range(2)]
                    for nh in range(2):
                        nc.tensor.matmul(
                            pouts[nh], attnT[:, 7, rti * 128:(rti + 1) * 128],
                            wo_sb[:, 7, nh * 512:(nh + 1) * 512],
                            start=False, stop=True)
                else:
                    pouts = [psum.tile([128, 512], F32, tag="work",
                                       name="pout") for _ in range(2)]
                    for ct2 in range(8):
                        for nh in range(2):
                            nc.tensor.matmul(
                                pouts[nh],
                                attnT[:, ct2, rti * 128:(rti + 1) * 128],
                                wo_sb[:, ct2, nh * 512:(nh + 1) * 512],
                                start=(ct2 == 0), stop=(ct2 == 7))
                for nh in range(2):
                    out_t = outpool.tile([128, 512], BF16, tag="out",
                                         name="out_t")
                    nc.vector.tensor_add(out_t, pouts[nh],
                                         bo_bc[:, nh * 512:(nh + 1) * 512])
                    nc.sync.dma_start(
                        out=out.ap()[rti * 128:(rti + 1) * 128,
                                     nh * 512:(nh + 1) * 512],
                        in_=out_t)
            attnstack.close()

    nc.compile()
    return nc


def _swz(a):
    """[1024, cols] -> [128, 8, cols] with row r -> (r % 128, r // 128)."""
    return np.ascontiguousarray(
        a.reshape(8, 128, -1).transpose(1, 0, 2))


def _make_in_maps(x, Wq, bq, Wk, bk, Wv, bv, Wqt, bqt, Wo, bo):
    x = np.asarray(x, dtype=np.float32)
    bf = ml_dtypes.bfloat16

    wq_b = np.ascontiguousarray(np.asarray(Wq, np.float32).astype(bf))
    wqt_b = np.ascontiguousarray(np.asarray(Wqt, np.float32).astype(bf))
    wk_b = np.ascontiguousarray(np.asarray(Wk, np.float32).astype(bf))
    wv_b = np.ascontiguousarray(np.asarray(Wv, np.float32).astype(bf))
    wo_b = np.ascontiguousarray(np.asarray(Wo, np.float32).astype(bf))
    colmaj = lambda v: np.ascontiguousarray(
        np.asarray(v, np.float32).reshape(8, 128).T)
    bq, bqt = map(colmaj, (bq, bqt))
    # bv folded through Wo into the output bias (softmax rows sum to 1);
    # bk dropped (constant score shift per row, softmax-invariant).
    bo_eff = (np.asarray(bo, np.float32)
              + np.asarray(bv, np.float32) @ np.asarray(Wo, np.float32))
    bo_eff = np.ascontiguousarray(np.broadcast_to(bo_eff, (128, D)))

    wq_sw, wqt_sw = _swz(wq_b), _swz(wqt_b)
    pairmaj = lambda w: np.ascontiguousarray(
        _swz(w).reshape(128, 8, 8, 128).transpose(0, 2, 1, 3))
    wk_sw, wv_sw, wo_sw = pairmaj(wk_b), pairmaj(wv_b), _swz(wo_b)
    at_sw = [_swz(x[b, :A, :].T.astype(bf)) for b in range(B)]
    in_maps = []
    for c in range(NCORES):
        b, q = divmod(c, 4)
        rows = x[b, q * RPC:(q + 1) * RPC, :]
        in_maps.append({
            "xt": _swz(rows.T.astype(bf)),
            "at": at_sw[b],
            "wlo": wq_sw if q == 0 else wqt_sw,
            "whi": wqt_sw,
            "wk": wk_sw, "wv": wv_sw, "wo": wo_sw,
            "blo": bq if q == 0 else bqt, "bhi": bqt,
            "bo": bo_eff,
        })
    return in_maps


def kernel(x, Wq, bq, Wk, bk, Wv, bv, Wqt, bqt, Wo, bo, num_anchor_tokens):
    assert int(num_anchor_tokens) == A
    if "nc" not in _CACHE:
        _CACHE["nc"] = _build()
    nc = _CACHE["nc"]

    in_maps = _make_in_maps(x, Wq, bq, Wk, bk, Wv, bv, Wqt, bqt, Wo, bo)
    res = bass_utils.run_bass_kernel_spmd(
        nc, in_maps, core_ids=list(range(NCORES)))
    out = np.empty((B, S, D), np.float32)
    for c in range(NCORES):
        b, q = divmod(c, 4)
        out[b, q * RPC:(q + 1) * RPC, :] = res.results[c]["out"].astype(
            np.float32)
    return out
